# revision 7
# baseline (speedup 1.0000x reference)
"""Trainium2 Bass kernel for ConsistentSelfAttentionTile.

Reference semantics: T=449 overlapping 64-token tiles; each tile attends to
352 KV tokens = 288 sampled (from a 9x replication of the tile) + the tile
itself; outputs overlap-add, then divide by overlap counts.

Algebraic collapse used here (verified to ~1e-6 rel vs the jax reference):
  * rep[:, idx, :] == tile[:, idx % 64, :], so the sampled KV tokens are tile
    rows with integer multiplicities m_t[w] = 1 + #{s : idx[t,s] % 64 == w}.
  * Per-tile Q/K/V are slices of the full-sequence projections, so all
    per-tile 64x64 score blocks are diagonal blocks of one banded 512x512
    score matrix S = Q K^T (band |i-j| <= 63).
  * With E = exp(S - rowmax), Cm[j,t] = m_t[j-t] (banded), the full
    tile-softmax + overlap-add + count-divide collapses to
        Z = E @ Cm;  W = bandmask/(counts * Z);  U = W @ Cm^T;
        out = (E * U) @ V
    i.e. three extra banded 512x512 matmuls instead of 449 gathered
    attentions.

Sharding: 8 cores = 2 batches x 4 row-chunks of 128 output rows. Each core
computes its 128 rows end-to-end from a 256-column band of the input (no
cross-core communication); host slices/pads inputs and concatenates outputs.

All per-core inputs are packed host-side into ONE [128, F] fp32 blob laid
out exactly as the SBUF destination -> a single input DMA (one DMA-queue
semaphore; PE matmuls may carry only one sync wait).
"""

import os
import sys

import numpy as np

for _p in ("/opt/trn_rl_repo",):
    if _p not in sys.path and os.path.isdir(_p):
        sys.path.insert(0, _p)

B, N, C, W = 2, 512, 512, 64
T = N - W + 1          # 449 tiles
RCH = 128              # output rows per core
NCORES = 8
BAND = 256             # per-core j/t band width (columns [r0-64, r0+192))
KC = C // 128          # 4 contraction chunks
JC = BAND // 128       # 2 band chunks

# blob layout (fp32 elements per partition)
OFF_XT = 0                       # [128, 4, 256]
OFF_WQT = OFF_XT + KC * BAND     # [128, 4, 512]
OFF_WKT = OFF_WQT + KC * C
OFF_WVT = OFF_WKT + KC * C
OFF_BQ = OFF_WVT + KC * C        # [128, 4, 1]
OFF_BK = OFF_BQ + KC
OFF_BV = OFF_BK + KC             # [1, 512] (partition 0)
OFF_CM = OFF_BV + C              # [128, 2, 256]
OFF_CMT = OFF_CM + JC * BAND
OFF_MW = OFF_CMT + JC * BAND     # [128, 2, 128]
OFF_ID = OFF_MW + JC * RCH       # [128, 128]
OFF_ONES = OFF_ID + 128          # [1, 128] (partition 0)
FBLOB = OFF_ONES + 128

_CACHE = {}


def _build_program():
    import concourse.bacc as bacc
    import concourse.mybir as mybir
    import concourse.tile as tile

    fp32 = mybir.dt.float32
    nc = bacc.Bacc("TRN2", target_bir_lowering=False, debug=False)

    blob_d = nc.declare_dram_parameter("blob", [128, FBLOB], fp32, isOutput=False)
    out_d = nc.declare_dram_parameter("out", [RCH, C], fp32, isOutput=True)

    with tile.TileContext(nc) as tc:
        with (
            tc.tile_pool(name="consts", bufs=1) as consts,
            tc.tile_pool(name="work", bufs=1) as work,
            tc.tile_pool(name="psum", bufs=1, space="PSUM") as psum,
        ):
            blob = consts.tile([128, FBLOB], fp32)
            nc.sync.dma_start(out=blob, in_=blob_d[:])

            def seg(off, length):
                return blob[:, off:off + length]

            xt_sb = seg(OFF_XT, KC * BAND).rearrange("p (k j) -> p k j", k=KC)
            wqt_sb = seg(OFF_WQT, KC * C).rearrange("p (k j) -> p k j", k=KC)
            wkt_sb = seg(OFF_WKT, KC * C).rearrange("p (k j) -> p k j", k=KC)
            wvt_sb = seg(OFF_WVT, KC * C).rearrange("p (k j) -> p k j", k=KC)
            bqc_sb = seg(OFF_BQ, KC).rearrange("p (k o) -> p k o", k=KC)
            bkc_sb = seg(OFF_BK, KC).rearrange("p (k o) -> p k o", k=KC)
            bvr_sb = blob[0:1, OFF_BV:OFF_BV + C]
            cm_sb = seg(OFF_CM, JC * BAND).rearrange("p (k t) -> p k t", k=JC)
            cmt_sb = seg(OFF_CMT, JC * BAND).rearrange("p (k j) -> p k j", k=JC)
            mw_sb = seg(OFF_MW, JC * RCH).rearrange("p (k r) -> p k r", k=JC)
            ident = seg(OFF_ID, 128)
            ones1 = blob[0:1, OFF_ONES:OFF_ONES + 128]

            # ---- projections ----
            # QT[m][c_out 128, r 128]: rows r = band cols [64, 192)
            qt_sb = work.tile([128, KC, RCH], fp32)
            for m in range(KC):
                ps_q = psum.tile([128, RCH], fp32, tag="ps_q", bufs=1)
                for k in range(KC):
                    nc.tensor.matmul(
                        ps_q,
                        lhsT=wqt_sb[:, k, m * 128:(m + 1) * 128],
                        rhs=xt_sb[:, k, 64:64 + RCH],
                        start=(k == 0),
                        stop=(k == KC - 1),
                    )
                # copy + per-partition bias add
                nc.scalar.activation(
                    out=qt_sb[:, m, :], in_=ps_q,
                    func=mybir.ActivationFunctionType.Identity,
                    bias=bqc_sb[:, m, :], scale=1.0,
                )

            # KT[m][c_out 128, j 256]
            kt_sb = work.tile([128, KC, BAND], fp32)
            for m in range(KC):
                ps_k = psum.tile([128, BAND], fp32, tag="ps_k", bufs=1)
                for k in range(KC):
                    nc.tensor.matmul(
                        ps_k,
                        lhsT=wkt_sb[:, k, m * 128:(m + 1) * 128],
                        rhs=xt_sb[:, k, :],
                        start=(k == 0),
                        stop=(k == KC - 1),
                    )
                nc.scalar.activation(
                    out=kt_sb[:, m, :], in_=ps_k,
                    func=mybir.ActivationFunctionType.Identity,
                    bias=bkc_sb[:, m, :], scale=1.0,
                )

            # V[jc][j 128, c 512] (+bv via rank-1 ones matmul)
            v_sb = work.tile([128, JC, C], fp32)
            for jc in range(JC):
                ps_v = psum.tile([128, C], fp32, tag="ps_v", bufs=1)
                for k in range(KC):
                    nc.tensor.matmul(
                        ps_v,
                        lhsT=xt_sb[:, k, jc * 128:(jc + 1) * 128],
                        rhs=wvt_sb[:, k, :],
                        start=(k == 0),
                        stop=False,
                    )
                nc.tensor.matmul(
                    ps_v, lhsT=ones1, rhs=bvr_sb,
                    start=False, stop=True,
                )
                nc.vector.tensor_copy(out=v_sb[:, jc, :], in_=ps_v)

            # ---- scores and softmax numerator ----
            ps_s = psum.tile([128, BAND], fp32, tag="ps_s", bufs=1)
            for k in range(KC):
                nc.tensor.matmul(
                    ps_s,
                    lhsT=qt_sb[:, k, :],
                    rhs=kt_sb[:, k, :],
                    start=(k == 0),
                    stop=(k == KC - 1),
                )
            negmax = work.tile([128, 1], fp32)
            nc.vector.reduce_max(
                negmax, ps_s, axis=mybir.AxisListType.X, negate=True
            )
            e_sb = work.tile([128, BAND], fp32)
            nc.scalar.activation(
                out=e_sb, in_=ps_s,
                func=mybir.ActivationFunctionType.Exp,
                bias=negmax, scale=1.0,
            )

            # E^T chunks [j 128, r 128]
            et_sb = work.tile([128, JC, RCH], fp32)
            for jc in range(JC):
                ps_et = psum.tile([128, RCH], fp32, tag="ps_et", bufs=1)
                nc.tensor.transpose(
                    ps_et, e_sb[:, jc * 128:(jc + 1) * 128], ident
                )
                nc.vector.tensor_copy(out=et_sb[:, jc, :], in_=ps_et)

            # Z'[t 128, r 128] = sum_j Cm[j,t] E'[j,r];  W' = maskw / Z'
            w_sb = work.tile([128, JC, RCH], fp32)
            for tch in range(JC):
                ps_z = psum.tile([128, RCH], fp32, tag="ps_z", bufs=1)
                for jc in range(JC):
                    nc.tensor.matmul(
                        ps_z,
                        lhsT=cm_sb[:, jc, tch * 128:(tch + 1) * 128],
                        rhs=et_sb[:, jc, :],
                        start=(jc == 0),
                        stop=(jc == JC - 1),
                    )
                rz = work.tile([128, RCH], fp32, tag="rz", bufs=2)
                nc.vector.reciprocal(out=rz, in_=ps_z)
                nc.vector.tensor_mul(w_sb[:, tch, :], rz, mw_sb[:, tch, :])

            # U'[j 128, r 128] = sum_t Cm^T[t,j] W'[t,r];  A' = E' * U'
            a_sb = work.tile([128, JC, RCH], fp32)
            for jc in range(JC):
                ps_u = psum.tile([128, RCH], fp32, tag="ps_u", bufs=1)
                for tch in range(JC):
                    nc.tensor.matmul(
                        ps_u,
                        lhsT=cmt_sb[:, tch, jc * 128:(jc + 1) * 128],
                        rhs=w_sb[:, tch, :],
                        start=(tch == 0),
                        stop=(tch == JC - 1),
                    )
                nc.vector.tensor_mul(a_sb[:, jc, :], ps_u, et_sb[:, jc, :])

            # out rows [r 128, c 512] = sum_j A'[j,r]^T V[j,c]
            ps_o = psum.tile([128, C], fp32, tag="ps_o", bufs=1)
            for jc in range(JC):
                nc.tensor.matmul(
                    ps_o,
                    lhsT=a_sb[:, jc, :],
                    rhs=v_sb[:, jc, :],
                    start=(jc == 0),
                    stop=(jc == JC - 1),
                )
            o_sb = work.tile([128, C], fp32)
            nc.vector.tensor_copy(out=o_sb, in_=ps_o)
            nc.sync.dma_start(out=out_d[:], in_=o_sb)

    nc.compile()
    return nc


def _pack128(arr):
    """[n*128, f] row-chunked -> [128, n*f] (chunk-major along free axis)."""
    n = arr.shape[0] // 128
    return np.ascontiguousarray(
        arr.reshape(n, 128, -1).transpose(1, 0, 2).reshape(128, -1)
    )


def _host_prep(image_features, Wq, bq, Wk, bk, Wv, bv, sample_idx):
    """Build the 8 per-core input blobs (pure index/layout work)."""
    x = np.asarray(image_features, np.float32)
    sample_idx = np.asarray(sample_idx)

    # per-tile multiplicities -> banded count matrix Cm[j, t] = m_t[j - t]
    mod = (sample_idx % W).astype(np.int64)                  # [T, S]
    m = np.zeros((T, W), np.float32)
    np.add.at(m, (np.arange(T)[:, None], mod), 1.0)
    m += 1.0
    Cm = np.zeros((N, N), np.float32)
    rows = np.arange(T)
    for w in range(W):
        Cm[rows + w, rows] = m[:, w]

    pos = np.arange(N)
    counts = (np.minimum(pos, N - W) - np.maximum(pos - W + 1, 0) + 1)

    # padded versions for uniform band slicing
    XTp = np.zeros((B, C, N + 2 * 64), np.float32)
    for b in range(B):
        XTp[b, :, 64:64 + N] = x[b].T
    Cmp = np.zeros((N + 2 * 64, N + 2 * 64), np.float32)
    Cmp[64:64 + N, 64:64 + N] = Cm

    wqt_p = _pack128(np.asarray(Wq, np.float32).T)
    wkt_p = _pack128(np.asarray(Wk, np.float32).T)
    wvt_p = _pack128(np.asarray(Wv, np.float32).T)
    bq_p = _pack128(np.asarray(bq, np.float32).reshape(C, 1))
    bk_p = _pack128(np.asarray(bk, np.float32).reshape(C, 1))

    in_maps = []
    for core in range(NCORES):
        b, rc = divmod(core, NCORES // B)
        r0 = rc * RCH
        xt = XTp[b, :, r0:r0 + BAND]
        cm = np.ascontiguousarray(Cmp[r0:r0 + BAND, r0:r0 + BAND])
        # all-zero columns (padded t) would give Z=0 -> 1/0*mask = NaN on
        # device; a diagonal 1 keeps Z finite there and is masked out of W
        zero_cols = ~cm.any(axis=0)
        cm[zero_cols, zero_cols] = 1.0
        tl = np.arange(BAND)
        rl = np.arange(RCH)
        tg = r0 - 64 + tl
        rg = r0 + rl
        d = rg[None, :] - tg[:, None]
        valid = (d >= 0) & (d <= W - 1) & (tg[:, None] >= 0) & (tg[:, None] <= T - 1)
        maskw = np.where(
            valid, 1.0 / counts[rg][None, :], 0.0
        ).astype(np.float32)

        blob = np.zeros((128, FBLOB), np.float32)
        blob[:, OFF_XT:OFF_XT + KC * BAND] = _pack128(xt)
        blob[:, OFF_WQT:OFF_WQT + KC * C] = wqt_p
        blob[:, OFF_WKT:OFF_WKT + KC * C] = wkt_p
        blob[:, OFF_WVT:OFF_WVT + KC * C] = wvt_p
        blob[:, OFF_BQ:OFF_BQ + KC] = bq_p
        blob[:, OFF_BK:OFF_BK + KC] = bk_p
        blob[0, OFF_BV:OFF_BV + C] = np.asarray(bv, np.float32)
        blob[:, OFF_CM:OFF_CM + JC * BAND] = _pack128(cm)
        blob[:, OFF_CMT:OFF_CMT + JC * BAND] = _pack128(
            np.ascontiguousarray(cm.T)
        )
        blob[:, OFF_MW:OFF_MW + JC * RCH] = _pack128(maskw)
        blob[:, OFF_ID:OFF_ID + 128] = np.eye(128, dtype=np.float32)
        blob[0, OFF_ONES:OFF_ONES + 128] = 1.0
        in_maps.append({"blob": blob})
    return in_maps


def run_on_cores(in_maps, trace=False):
    from concourse.bass_utils import run_bass_kernel_spmd

    if "nc" not in _CACHE:
        _CACHE["nc"] = _build_program()
    nc = _CACHE["nc"]
    return run_bass_kernel_spmd(
        nc, in_maps, list(range(NCORES)), trace=trace,
        trace_cores=list(range(NCORES)) if trace else None,
    )


def kernel(image_features, Wq, bq, Wk, bk, Wv, bv, sample_idx):
    in_maps = _host_prep(image_features, Wq, bq, Wk, bk, Wv, bv, sample_idx)
    res = run_on_cores(in_maps, trace=False)
    out = np.empty((B, N, C), np.float32)
    for core in range(NCORES):
        b, rc = divmod(core, NCORES // B)
        out[b, rc * RCH:(rc + 1) * RCH, :] = res.results[core]["out"]
    return out


# revision 8
# speedup vs baseline: 1.0322x; 1.0322x over previous
"""Trainium2 Bass kernel for ConsistentSelfAttentionTile.

Reference semantics: T=449 overlapping 64-token tiles; each tile attends to
352 KV tokens = 288 sampled (from a 9x replication of the tile) + the tile
itself; outputs overlap-add, then divide by overlap counts.

Algebraic collapse used here (verified to ~1e-6 rel vs the jax reference):
  * rep[:, idx, :] == tile[:, idx % 64, :], so the sampled KV tokens are tile
    rows with integer multiplicities m_t[w] = 1 + #{s : idx[t,s] % 64 == w}.
  * Per-tile Q/K/V are slices of the full-sequence projections, so all
    per-tile 64x64 score blocks are diagonal blocks of one banded 512x512
    score matrix S = Q K^T (band |i-j| <= 63).
  * With E = exp(S - rowmax), Cm[j,t] = m_t[j-t] (banded), the full
    tile-softmax + overlap-add + count-divide collapses to
        Z = E @ Cm;  W = bandmask/(counts * Z);  U = W @ Cm^T;
        out = (E * U) @ V
    i.e. three extra banded 512x512 matmuls instead of 449 gathered
    attentions.

Sharding: 8 cores = 2 batches x 4 row-chunks of 128 output rows. Each core
computes its 128 rows end-to-end from a 256-column band of the input (no
cross-core communication); host slices/pads inputs and concatenates outputs.

All per-core inputs are packed host-side into ONE [128, F] fp32 blob laid
out exactly as the SBUF destination -> a single input DMA (one DMA-queue
semaphore; PE matmuls may carry only one sync wait).
"""

import os
import sys

import numpy as np

for _p in ("/opt/trn_rl_repo",):
    if _p not in sys.path and os.path.isdir(_p):
        sys.path.insert(0, _p)

B, N, C, W = 2, 512, 512, 64
T = N - W + 1          # 449 tiles
RCH = 128              # output rows per core
NCORES = 8
BAND = 256             # per-core j/t band width (columns [r0-64, r0+192))
KC = C // 128          # 4 contraction chunks
JC = BAND // 128       # 2 band chunks

# blob layout (fp32 elements per partition)
OFF_XT = 0                       # [128, 4, 256]
OFF_WQT = OFF_XT + KC * BAND     # [128, 4, 512]
OFF_WKT = OFF_WQT + KC * C
OFF_WVT = OFF_WKT + KC * C
OFF_BQ = OFF_WVT + KC * C        # [128, 4, 1]
OFF_BK = OFF_BQ + KC
OFF_BV = OFF_BK + KC             # [1, 512] (partition 0)
OFF_CM = OFF_BV + C              # [128, 2, 256]
OFF_CMT = OFF_CM + JC * BAND
OFF_MW = OFF_CMT + JC * BAND     # [128, 2, 128]
OFF_ID = OFF_MW + JC * RCH       # [128, 128]
OFF_ONES = OFF_ID + 128          # [1, 128] (partition 0)
FBLOB = OFF_ONES + 128

_CACHE = {}


def _build_program():
    import concourse.bacc as bacc
    import concourse.mybir as mybir
    import concourse.tile as tile

    fp32 = mybir.dt.float32
    nc = bacc.Bacc("TRN2", target_bir_lowering=False, debug=False)

    blob_d = nc.declare_dram_parameter("blob", [128, FBLOB], fp32, isOutput=False)
    out_d = nc.declare_dram_parameter("out", [RCH, C], fp32, isOutput=True)

    with tile.TileContext(nc) as tc:
        with (
            tc.tile_pool(name="consts", bufs=1) as consts,
            tc.tile_pool(name="work", bufs=1) as work,
            tc.tile_pool(name="psum", bufs=1, space="PSUM") as psum,
        ):
            blob = consts.tile([128, FBLOB], fp32)
            nc.sync.dma_start(out=blob, in_=blob_d[:])

            def seg(off, length):
                return blob[:, off:off + length]

            xt_sb = seg(OFF_XT, KC * BAND).rearrange("p (k j) -> p k j", k=KC)
            wqt_sb = seg(OFF_WQT, KC * C).rearrange("p (k j) -> p k j", k=KC)
            wkt_sb = seg(OFF_WKT, KC * C).rearrange("p (k j) -> p k j", k=KC)
            wvt_sb = seg(OFF_WVT, KC * C).rearrange("p (k j) -> p k j", k=KC)
            bqc_sb = seg(OFF_BQ, KC).rearrange("p (k o) -> p k o", k=KC)
            bkc_sb = seg(OFF_BK, KC).rearrange("p (k o) -> p k o", k=KC)
            bvr_sb = blob[0:1, OFF_BV:OFF_BV + C]
            cm_sb = seg(OFF_CM, JC * BAND).rearrange("p (k t) -> p k t", k=JC)
            cmt_sb = seg(OFF_CMT, JC * BAND).rearrange("p (k j) -> p k j", k=JC)
            mw_sb = seg(OFF_MW, JC * RCH).rearrange("p (k r) -> p k r", k=JC)
            ident = seg(OFF_ID, 128)
            ones1 = blob[0:1, OFF_ONES:OFF_ONES + 128]

            # ---- projections ----
            # QT[m][c_out 128, r 128]: rows r = band cols [64, 192)
            qt_sb = work.tile([128, KC, RCH], fp32)
            for m in range(KC):
                ps_q = psum.tile([128, RCH], fp32, tag="ps_q", bufs=1)
                for k in range(KC):
                    nc.tensor.matmul(
                        ps_q,
                        lhsT=wqt_sb[:, k, m * 128:(m + 1) * 128],
                        rhs=xt_sb[:, k, 64:64 + RCH],
                        start=(k == 0),
                        stop=(k == KC - 1),
                    )
                # copy + per-partition bias add
                nc.scalar.activation(
                    out=qt_sb[:, m, :], in_=ps_q,
                    func=mybir.ActivationFunctionType.Identity,
                    bias=bqc_sb[:, m, :], scale=1.0,
                )

            # KT[m][c_out 128, j 256]
            kt_sb = work.tile([128, KC, BAND], fp32)
            for m in range(KC):
                ps_k = psum.tile([128, BAND], fp32, tag="ps_k", bufs=1)
                for k in range(KC):
                    nc.tensor.matmul(
                        ps_k,
                        lhsT=wkt_sb[:, k, m * 128:(m + 1) * 128],
                        rhs=xt_sb[:, k, :],
                        start=(k == 0),
                        stop=(k == KC - 1),
                    )
                nc.scalar.activation(
                    out=kt_sb[:, m, :], in_=ps_k,
                    func=mybir.ActivationFunctionType.Identity,
                    bias=bkc_sb[:, m, :], scale=1.0,
                )

            # V[jc][j 128, c 512] (+bv via rank-1 ones matmul)
            v_sb = work.tile([128, JC, C], fp32)
            for jc in range(JC):
                ps_v = psum.tile([128, C], fp32, tag="ps_v", bufs=1)
                for k in range(KC):
                    nc.tensor.matmul(
                        ps_v,
                        lhsT=xt_sb[:, k, jc * 128:(jc + 1) * 128],
                        rhs=wvt_sb[:, k, :],
                        start=(k == 0),
                        stop=False,
                    )
                nc.tensor.matmul(
                    ps_v, lhsT=ones1, rhs=bvr_sb,
                    start=False, stop=True,
                )
                nc.vector.tensor_copy(out=v_sb[:, jc, :], in_=ps_v)

            # ---- scores and softmax numerator ----
            ps_s = psum.tile([128, BAND], fp32, tag="ps_s", bufs=1)
            for k in range(KC):
                nc.tensor.matmul(
                    ps_s,
                    lhsT=qt_sb[:, k, :],
                    rhs=kt_sb[:, k, :],
                    start=(k == 0),
                    stop=(k == KC - 1),
                )
            negmax = work.tile([128, 1], fp32)
            nc.vector.reduce_max(
                negmax, ps_s, axis=mybir.AxisListType.X, negate=True
            )
            e_sb = work.tile([128, BAND], fp32)
            nc.scalar.activation(
                out=e_sb, in_=ps_s,
                func=mybir.ActivationFunctionType.Exp,
                bias=negmax, scale=1.0,
            )

            # E^T chunks [j 128, r 128]
            et_sb = work.tile([128, JC, RCH], fp32)
            for jc in range(JC):
                ps_et = psum.tile([128, RCH], fp32, tag="ps_et", bufs=1)
                nc.tensor.transpose(
                    ps_et, e_sb[:, jc * 128:(jc + 1) * 128], ident
                )
                nc.vector.tensor_copy(out=et_sb[:, jc, :], in_=ps_et)

            # Z'[t 128, r 128] = sum_j Cm[j,t] E'[j,r];  W' = maskw / Z'
            w_sb = work.tile([128, JC, RCH], fp32)
            for tch in range(JC):
                ps_z = psum.tile([128, RCH], fp32, tag="ps_z", bufs=1)
                for jc in range(JC):
                    nc.tensor.matmul(
                        ps_z,
                        lhsT=cm_sb[:, jc, tch * 128:(tch + 1) * 128],
                        rhs=et_sb[:, jc, :],
                        start=(jc == 0),
                        stop=(jc == JC - 1),
                    )
                rz = work.tile([128, RCH], fp32, tag="rz", bufs=2)
                nc.vector.reciprocal(out=rz, in_=ps_z)
                nc.vector.tensor_mul(w_sb[:, tch, :], rz, mw_sb[:, tch, :])

            # U'[j 128, r 128] = sum_t Cm^T[t,j] W'[t,r];  A' = E' * U'
            a_sb = work.tile([128, JC, RCH], fp32)
            for jc in range(JC):
                ps_u = psum.tile([128, RCH], fp32, tag="ps_u", bufs=1)
                for tch in range(JC):
                    nc.tensor.matmul(
                        ps_u,
                        lhsT=cmt_sb[:, tch, jc * 128:(jc + 1) * 128],
                        rhs=w_sb[:, tch, :],
                        start=(tch == 0),
                        stop=(tch == JC - 1),
                    )
                nc.vector.tensor_mul(a_sb[:, jc, :], ps_u, et_sb[:, jc, :])

            # out rows [r 128, c 512] = sum_j A'[j,r]^T V[j,c]
            ps_o = psum.tile([128, C], fp32, tag="ps_o", bufs=1)
            for jc in range(JC):
                nc.tensor.matmul(
                    ps_o,
                    lhsT=a_sb[:, jc, :],
                    rhs=v_sb[:, jc, :],
                    start=(jc == 0),
                    stop=(jc == JC - 1),
                )
            o_sb = work.tile([128, C], fp32)
            nc.vector.tensor_copy(out=o_sb, in_=ps_o)
            nc.sync.dma_start(out=out_d[:], in_=o_sb)

    nc.compile()
    return nc


def _pack128(arr):
    """[n*128, f] row-chunked -> [128, n*f] (chunk-major along free axis)."""
    n = arr.shape[0] // 128
    return np.ascontiguousarray(
        arr.reshape(n, 128, -1).transpose(1, 0, 2).reshape(128, -1)
    )


def _host_prep(image_features, Wq, bq, Wk, bk, Wv, bv, sample_idx):
    """Build the 8 per-core input blobs (pure index/layout work)."""
    x = np.asarray(image_features, np.float32)
    sample_idx = np.asarray(sample_idx)

    # per-tile multiplicities -> banded count matrix Cm[j, t] = m_t[j - t]
    mod = (sample_idx % W).astype(np.int64)                  # [T, S]
    m = np.zeros((T, W), np.float32)
    np.add.at(m, (np.arange(T)[:, None], mod), 1.0)
    m += 1.0
    Cm = np.zeros((N, N), np.float32)
    rows = np.arange(T)
    for w in range(W):
        Cm[rows + w, rows] = m[:, w]

    pos = np.arange(N)
    counts = (np.minimum(pos, N - W) - np.maximum(pos - W + 1, 0) + 1)

    # padded versions for uniform band slicing
    XTp = np.zeros((B, C, N + 2 * 64), np.float32)
    for b in range(B):
        XTp[b, :, 64:64 + N] = x[b].T
    Cmp = np.zeros((N + 2 * 64, N + 2 * 64), np.float32)
    Cmp[64:64 + N, 64:64 + N] = Cm

    wqt_p = _pack128(np.asarray(Wq, np.float32).T)
    wkt_p = _pack128(np.asarray(Wk, np.float32).T)
    wvt_p = _pack128(np.asarray(Wv, np.float32).T)
    bq_p = _pack128(np.asarray(bq, np.float32).reshape(C, 1))
    bk_p = _pack128(np.asarray(bk, np.float32).reshape(C, 1))

    in_maps = []
    for core in range(NCORES):
        b, rc = divmod(core, NCORES // B)
        r0 = rc * RCH
        xt = XTp[b, :, r0:r0 + BAND]
        cm = np.ascontiguousarray(Cmp[r0:r0 + BAND, r0:r0 + BAND])
        # all-zero columns (padded t) would give Z=0 -> 1/0*mask = NaN on
        # device; a diagonal 1 keeps Z finite there and is masked out of W
        zero_cols = ~cm.any(axis=0)
        cm[zero_cols, zero_cols] = 1.0
        tl = np.arange(BAND)
        rl = np.arange(RCH)
        tg = r0 - 64 + tl
        rg = r0 + rl
        d = rg[None, :] - tg[:, None]
        valid = (d >= 0) & (d <= W - 1) & (tg[:, None] >= 0) & (tg[:, None] <= T - 1)
        maskw = np.where(
            valid, 1.0 / counts[rg][None, :], 0.0
        ).astype(np.float32)

        blob = np.zeros((128, FBLOB), np.float32)
        blob[:, OFF_XT:OFF_XT + KC * BAND] = _pack128(xt)
        blob[:, OFF_WQT:OFF_WQT + KC * C] = wqt_p
        blob[:, OFF_WKT:OFF_WKT + KC * C] = wkt_p
        blob[:, OFF_WVT:OFF_WVT + KC * C] = wvt_p
        blob[:, OFF_BQ:OFF_BQ + KC] = bq_p
        blob[:, OFF_BK:OFF_BK + KC] = bk_p
        blob[0, OFF_BV:OFF_BV + C] = np.asarray(bv, np.float32)
        blob[:, OFF_CM:OFF_CM + JC * BAND] = _pack128(cm)
        blob[:, OFF_CMT:OFF_CMT + JC * BAND] = _pack128(
            np.ascontiguousarray(cm.T)
        )
        blob[:, OFF_MW:OFF_MW + JC * RCH] = _pack128(maskw)
        blob[:, OFF_ID:OFF_ID + 128] = np.eye(128, dtype=np.float32)
        blob[0, OFF_ONES:OFF_ONES + 128] = 1.0
        in_maps.append({"blob": blob})
    return in_maps


def run_on_cores(in_maps, trace=False, trace_cores=None):
    from concourse.bass_utils import run_bass_kernel_spmd

    if "nc" not in _CACHE:
        _CACHE["nc"] = _build_program()
    nc = _CACHE["nc"]
    return run_bass_kernel_spmd(
        nc, in_maps, list(range(NCORES)), trace=trace,
        trace_cores=(trace_cores or [0]) if trace else None,
    )


def kernel(image_features, Wq, bq, Wk, bk, Wv, bv, sample_idx):
    in_maps = _host_prep(image_features, Wq, bq, Wk, bk, Wv, bv, sample_idx)
    res = run_on_cores(in_maps, trace=False)
    out = np.empty((B, N, C), np.float32)
    for core in range(NCORES):
        b, rc = divmod(core, NCORES // B)
        out[b, rc * RCH:(rc + 1) * RCH, :] = res.results[core]["out"]
    return out


# revision 10
# speedup vs baseline: 1.2194x; 1.1814x over previous
"""Trainium2 Bass kernel for ConsistentSelfAttentionTile.

Reference semantics: T=449 overlapping 64-token tiles; each tile attends to
352 KV tokens = 288 sampled (from a 9x replication of the tile) + the tile
itself; outputs overlap-add, then divide by overlap counts.

Algebraic collapse used here (verified to ~1e-6 rel vs the jax reference):
  * rep[:, idx, :] == tile[:, idx % 64, :], so the sampled KV tokens are tile
    rows with integer multiplicities m_t[w] = 1 + #{s : idx[t,s] % 64 == w}.
  * Per-tile Q/K/V are slices of the full-sequence projections, so all
    per-tile 64x64 score blocks are diagonal blocks of one banded 512x512
    score matrix S = Q K^T (band |i-j| <= 63).
  * With E = exp(S - rowmax), Cm[j,t] = m_t[j-t] (banded), the full
    tile-softmax + overlap-add + count-divide collapses to
        Z = E @ Cm;  W = bandmask/(counts * Z);  U = W @ Cm^T;
        out = (E * U) @ V
    i.e. three extra banded 512x512 matmuls instead of 449 gathered
    attentions.
  * bk drops exactly: it shifts each row's scores by a constant, which the
    rowmax-subtracted softmax cancels bit-for-bit.

Sharding: 8 cores = 2 batches x 4 row-chunks of 128 output rows. Each core
computes its 128 rows end-to-end from a 256-column band of the input (no
cross-core communication); host slices/pads inputs and concatenates outputs.

Matmuls run in float32r (4x faster than fp32 at free-dim >= 256; ~13-bit
mantissa, measured 1.5e-4 rel err per 128-deep dot).

All per-core inputs are packed host-side into ONE [128, F] fp32 blob laid
out exactly as the SBUF destination -> a single input DMA (one DMA-queue
semaphore; instructions may carry only one sync wait).
"""

import os
import sys

import numpy as np

for _p in ("/opt/trn_rl_repo",):
    if _p not in sys.path and os.path.isdir(_p):
        sys.path.insert(0, _p)

B, N, C, W = 2, 512, 512, 64
T = N - W + 1          # 449 tiles
RCH = 128              # output rows per core
NCORES = 8
BAND = 256             # per-core j/t band width (columns [r0-64, r0+192))
KC = C // 128          # 4 contraction chunks
JC = BAND // 128       # 2 band chunks

# blob layout (fp32 elements per partition)
OFF_XT = 0                       # [128, 4, 256]
OFF_WQT = OFF_XT + KC * BAND     # [128, 4, 512]
OFF_WKT = OFF_WQT + KC * C
OFF_WVT = OFF_WKT + KC * C
OFF_BQR = OFF_WVT + KC * C       # [1, 512] (partition 0)
OFF_BV = OFF_BQR + C             # [1, 512] (partition 0)
OFF_CM = OFF_BV + C              # [128, 2, 256]
OFF_CMT = OFF_CM + JC * BAND
OFF_MW = OFF_CMT + JC * BAND     # [128, 2, 128]
OFF_ID = OFF_MW + JC * RCH       # [128, 128]
OFF_ONES = OFF_ID + 128          # [1, 128] (partition 0)
FBLOB = OFF_ONES + 128

_CACHE = {}


def _build_program():
    import concourse.bacc as bacc
    import concourse.mybir as mybir
    import concourse.tile as tile

    fp32 = mybir.dt.float32
    fp32r = mybir.dt.float32r
    nc = bacc.Bacc("TRN2", target_bir_lowering=False, debug=False)

    blob_d = nc.declare_dram_parameter("blob", [128, FBLOB], fp32, isOutput=False)
    out_d = nc.declare_dram_parameter("out", [RCH, C], fp32, isOutput=True)

    with tile.TileContext(nc) as tc:
        with (
            tc.tile_pool(name="consts", bufs=1) as consts,
            tc.tile_pool(name="work", bufs=1) as work,
            tc.tile_pool(name="psum", bufs=1, space="PSUM") as psum,
        ):
            # blob is typed float32r so matmuls can consume it directly;
            # non-matmul users view it as fp32 (same bits).
            blob = consts.tile([128, FBLOB], fp32r)
            nc.sync.dma_start(out=blob, in_=blob_d[:].bitcast(fp32r))

            def seg(off, length):
                return blob[:, off:off + length]

            xt_sb = seg(OFF_XT, KC * BAND).rearrange("p (k j) -> p k j", k=KC)
            wqt_sb = seg(OFF_WQT, KC * C).rearrange("p (k j) -> p k j", k=KC)
            wkt_sb = seg(OFF_WKT, KC * C).rearrange("p (k j) -> p k j", k=KC)
            wvt_sb = seg(OFF_WVT, KC * C).rearrange("p (k j) -> p k j", k=KC)
            bqr_sb = blob[0:1, OFF_BQR:OFF_BQR + C]
            bvr_sb = blob[0:1, OFF_BV:OFF_BV + C]
            cm_sb = seg(OFF_CM, JC * BAND).rearrange("p (k t) -> p k t", k=JC)
            cmt_sb = seg(OFF_CMT, JC * BAND).rearrange("p (k j) -> p k j", k=JC)
            mw_sb = seg(OFF_MW, JC * RCH).rearrange("p (k r) -> p k r", k=JC)
            ident = seg(OFF_ID, 128)
            ones1 = blob[0:1, OFF_ONES:OFF_ONES + 128]

            # ---- projections ----
            # Q rows [r 128, c 512] (+bq via rank-1 ones matmul), then
            # transpose to QT chunks [c 128, r 128]
            ps_qrow = psum.tile([128, C], fp32, tag="ps_big", bufs=2)
            for k in range(KC):
                nc.tensor.matmul(
                    ps_qrow,
                    lhsT=xt_sb[:, k, 64:64 + RCH],
                    rhs=wqt_sb[:, k, :],
                    start=(k == 0),
                    stop=False,
                )
            nc.tensor.matmul(
                ps_qrow, lhsT=ones1, rhs=bqr_sb, start=False, stop=True,
            )
            q_sb = work.tile([128, C], fp32r)
            nc.vector.tensor_copy(out=q_sb, in_=ps_qrow)
            qt_sb = work.tile([128, KC, RCH], fp32r)
            for m in range(KC):
                ps_t = psum.tile([128, RCH], fp32r, tag="ps_t", bufs=1)
                nc.tensor.transpose(
                    ps_t, q_sb[:, m * 128:(m + 1) * 128], ident
                )
                nc.vector.tensor_copy(out=qt_sb[:, m, :], in_=ps_t)

            # KT[m][c_out 128, j 256]  (bk dropped: softmax-invariant)
            kt_sb = work.tile([128, KC, BAND], fp32r)
            for m in range(KC):
                ps_k = psum.tile([128, BAND], fp32, tag="ps_k", bufs=1)
                for k in range(KC):
                    nc.tensor.matmul(
                        ps_k,
                        lhsT=wkt_sb[:, k, m * 128:(m + 1) * 128],
                        rhs=xt_sb[:, k, :],
                        start=(k == 0),
                        stop=(k == KC - 1),
                    )
                nc.vector.tensor_copy(out=kt_sb[:, m, :], in_=ps_k)

            # V[jc][j 128, c 512] (+bv via rank-1 ones matmul)
            v_sb = work.tile([128, JC, C], fp32r)
            for jc in range(JC):
                ps_v = psum.tile([128, C], fp32, tag="ps_big", bufs=2)
                for k in range(KC):
                    nc.tensor.matmul(
                        ps_v,
                        lhsT=xt_sb[:, k, jc * 128:(jc + 1) * 128],
                        rhs=wvt_sb[:, k, :],
                        start=(k == 0),
                        stop=False,
                    )
                nc.tensor.matmul(
                    ps_v, lhsT=ones1, rhs=bvr_sb, start=False, stop=True,
                )
                nc.vector.tensor_copy(out=v_sb[:, jc, :], in_=ps_v)

            # ---- scores and softmax numerator ----
            ps_s = psum.tile([128, BAND], fp32, tag="ps_s", bufs=1)
            for k in range(KC):
                nc.tensor.matmul(
                    ps_s,
                    lhsT=qt_sb[:, k, :],
                    rhs=kt_sb[:, k, :],
                    start=(k == 0),
                    stop=(k == KC - 1),
                )
            negmax = work.tile([128, 1], fp32)
            nc.vector.reduce_max(
                negmax, ps_s, axis=mybir.AxisListType.X, negate=True
            )
            e_sb = work.tile([128, BAND], fp32r)
            nc.scalar.activation(
                out=e_sb, in_=ps_s,
                func=mybir.ActivationFunctionType.Exp,
                bias=negmax, scale=1.0,
            )

            # E^T chunks [j 128, r 128]
            et_sb = work.tile([128, JC, RCH], fp32r)
            for jc in range(JC):
                ps_t = psum.tile([128, RCH], fp32r, tag="ps_t", bufs=1)
                nc.tensor.transpose(
                    ps_t, e_sb[:, jc * 128:(jc + 1) * 128], ident
                )
                nc.vector.tensor_copy(out=et_sb[:, jc, :], in_=ps_t)

            # Z'[t 128, r 128] = sum_j Cm[j,t] E'[j,r];  W' = maskw / Z'
            w_sb = work.tile([128, JC, RCH], fp32r)
            for tch in range(JC):
                ps_z = psum.tile([128, RCH], fp32, tag="ps_z", bufs=1)
                for jc in range(JC):
                    nc.tensor.matmul(
                        ps_z,
                        lhsT=cm_sb[:, jc, tch * 128:(tch + 1) * 128],
                        rhs=et_sb[:, jc, :],
                        start=(jc == 0),
                        stop=(jc == JC - 1),
                    )
                rz = work.tile([128, RCH], fp32, tag="rz", bufs=2)
                nc.vector.reciprocal(out=rz, in_=ps_z)
                nc.vector.tensor_mul(
                    w_sb[:, tch, :], rz, mw_sb[:, tch, :].bitcast(fp32)
                )

            # U'[j 128, r 128] = sum_t Cm^T[t,j] W'[t,r];  A' = E' * U'
            a_sb = work.tile([128, JC, RCH], fp32r)
            for jc in range(JC):
                ps_u = psum.tile([128, RCH], fp32, tag="ps_u", bufs=1)
                for tch in range(JC):
                    nc.tensor.matmul(
                        ps_u,
                        lhsT=cmt_sb[:, tch, jc * 128:(jc + 1) * 128],
                        rhs=w_sb[:, tch, :],
                        start=(tch == 0),
                        stop=(tch == JC - 1),
                    )
                nc.vector.tensor_mul(
                    a_sb[:, jc, :], ps_u, et_sb[:, jc, :].bitcast(fp32)
                )

            # out rows [r 128, c 512] = sum_j A'[j,r]^T V[j,c]
            ps_o = psum.tile([128, C], fp32, tag="ps_o", bufs=1)
            for jc in range(JC):
                nc.tensor.matmul(
                    ps_o,
                    lhsT=a_sb[:, jc, :],
                    rhs=v_sb[:, jc, :],
                    start=(jc == 0),
                    stop=(jc == JC - 1),
                )
            o_sb = work.tile([128, C], fp32)
            nc.vector.tensor_copy(out=o_sb, in_=ps_o)
            nc.sync.dma_start(out=out_d[:], in_=o_sb)

    nc.compile()
    return nc


def _pack128(arr):
    """[n*128, f] row-chunked -> [128, n*f] (chunk-major along free axis)."""
    n = arr.shape[0] // 128
    return np.ascontiguousarray(
        arr.reshape(n, 128, -1).transpose(1, 0, 2).reshape(128, -1)
    )


def _host_prep(image_features, Wq, bq, Wk, bk, Wv, bv, sample_idx):
    """Build the 8 per-core input blobs (pure index/layout work)."""
    x = np.asarray(image_features, np.float32)
    sample_idx = np.asarray(sample_idx)

    # per-tile multiplicities -> banded count matrix Cm[j, t] = m_t[j - t]
    mod = (sample_idx % W).astype(np.int64)                  # [T, S]
    m = np.zeros((T, W), np.float32)
    np.add.at(m, (np.arange(T)[:, None], mod), 1.0)
    m += 1.0
    Cm = np.zeros((N, N), np.float32)
    rows = np.arange(T)
    for w in range(W):
        Cm[rows + w, rows] = m[:, w]

    pos = np.arange(N)
    counts = (np.minimum(pos, N - W) - np.maximum(pos - W + 1, 0) + 1)

    # padded versions for uniform band slicing
    XTp = np.zeros((B, C, N + 2 * 64), np.float32)
    for b in range(B):
        XTp[b, :, 64:64 + N] = x[b].T
    Cmp = np.zeros((N + 2 * 64, N + 2 * 64), np.float32)
    Cmp[64:64 + N, 64:64 + N] = Cm

    wqt_p = _pack128(np.asarray(Wq, np.float32).T)
    wkt_p = _pack128(np.asarray(Wk, np.float32).T)
    wvt_p = _pack128(np.asarray(Wv, np.float32).T)

    in_maps = []
    for core in range(NCORES):
        b, rc = divmod(core, NCORES // B)
        r0 = rc * RCH
        xt = XTp[b, :, r0:r0 + BAND]
        cm = np.ascontiguousarray(Cmp[r0:r0 + BAND, r0:r0 + BAND])
        # all-zero columns (padded t) would give Z=0 -> 1/0*mask = NaN on
        # device; a diagonal 1 keeps Z finite there and is masked out of W
        zero_cols = ~cm.any(axis=0)
        cm[zero_cols, zero_cols] = 1.0
        tl = np.arange(BAND)
        rl = np.arange(RCH)
        tg = r0 - 64 + tl
        rg = r0 + rl
        d = rg[None, :] - tg[:, None]
        valid = (d >= 0) & (d <= W - 1) & (tg[:, None] >= 0) & (tg[:, None] <= T - 1)
        maskw = np.where(
            valid, 1.0 / counts[rg][None, :], 0.0
        ).astype(np.float32)

        blob = np.zeros((128, FBLOB), np.float32)
        blob[:, OFF_XT:OFF_XT + KC * BAND] = _pack128(xt)
        blob[:, OFF_WQT:OFF_WQT + KC * C] = wqt_p
        blob[:, OFF_WKT:OFF_WKT + KC * C] = wkt_p
        blob[:, OFF_WVT:OFF_WVT + KC * C] = wvt_p
        blob[0, OFF_BQR:OFF_BQR + C] = np.asarray(bq, np.float32)
        blob[0, OFF_BV:OFF_BV + C] = np.asarray(bv, np.float32)
        blob[:, OFF_CM:OFF_CM + JC * BAND] = _pack128(cm)
        blob[:, OFF_CMT:OFF_CMT + JC * BAND] = _pack128(
            np.ascontiguousarray(cm.T)
        )
        blob[:, OFF_MW:OFF_MW + JC * RCH] = _pack128(maskw)
        blob[:, OFF_ID:OFF_ID + 128] = np.eye(128, dtype=np.float32)
        blob[0, OFF_ONES:OFF_ONES + 128] = 1.0
        in_maps.append({"blob": blob})
    return in_maps


def run_on_cores(in_maps, trace=False, trace_cores=None):
    from concourse.bass_utils import run_bass_kernel_spmd

    if "nc" not in _CACHE:
        _CACHE["nc"] = _build_program()
    nc = _CACHE["nc"]
    return run_bass_kernel_spmd(
        nc, in_maps, list(range(NCORES)), trace=trace,
        trace_cores=(trace_cores or [0]) if trace else None,
    )


def kernel(image_features, Wq, bq, Wk, bk, Wv, bv, sample_idx):
    in_maps = _host_prep(image_features, Wq, bq, Wk, bk, Wv, bv, sample_idx)
    res = run_on_cores(in_maps, trace=False)
    out = np.empty((B, N, C), np.float32)
    for core in range(NCORES):
        b, rc = divmod(core, NCORES // B)
        out[b, rc * RCH:(rc + 1) * RCH, :] = res.results[core]["out"]
    return out


# revision 19
# speedup vs baseline: 1.2979x; 1.0643x over previous
"""Trainium2 Bass kernel for ConsistentSelfAttentionTile.

Reference semantics: T=449 overlapping 64-token tiles; each tile attends to
352 KV tokens = 288 sampled (from a 9x replication of the tile) + the tile
itself; outputs overlap-add, then divide by overlap counts.

Algebraic collapse used here (verified to ~1e-6 rel vs the jax reference):
  * rep[:, idx, :] == tile[:, idx % 64, :], so the sampled KV tokens are tile
    rows with integer multiplicities m_t[w] = 1 + #{s : idx[t,s] % 64 == w}.
  * Per-tile Q/K/V are slices of the full-sequence projections, so all
    per-tile 64x64 score blocks are diagonal blocks of one banded 512x512
    score matrix S = Q K^T (band |i-j| <= 63).
  * With E = exp(S - rowmax), Cm[j,t] = m_t[j-t] (banded), the full
    tile-softmax + overlap-add + count-divide collapses to
        Z = E @ Cm;  W = bandmask/(counts * Z);  U = W @ Cm^T;
        out = (E * U) @ V
    i.e. three extra banded 512x512 matmuls instead of 449 gathered
    attentions.
  * bk drops exactly: it shifts each row's scores by a constant, which the
    rowmax-subtracted softmax cancels bit-for-bit.

Sharding: 8 cores = 2 batches x 4 row-chunks of 128 output rows. Each core
computes its 128 rows end-to-end from a 256-column band of the input (no
cross-core communication); host slices/pads inputs and concatenates outputs.

Matmuls run in float32r (4x faster than fp32 at free-dim >= 256; ~13-bit
mantissa, measured 1.5e-4 rel err per 128-deep dot).

All per-core inputs are packed host-side into ONE [128, F] fp32 blob laid
out exactly as the SBUF destination -> a single input DMA (one DMA-queue
semaphore; instructions may carry only one sync wait).
"""

import os
import sys

import numpy as np

for _p in ("/opt/trn_rl_repo",):
    if _p not in sys.path and os.path.isdir(_p):
        sys.path.insert(0, _p)

B, N, C, W = 2, 512, 512, 64
T = N - W + 1          # 449 tiles
RCH = 128              # output rows per core
NCORES = 8
BAND = 256             # per-core j/t band width (columns [r0-64, r0+192))
KC = C // 128          # 4 contraction chunks
JC = BAND // 128       # 2 band chunks

# blob layout (fp32 elements per partition), ordered so the input DMA can be
# split into priority-chained pieces: [Q inputs | wkt | softmax consts | wvt]
OFF_XT = 0                       # [128, 4, 256]
OFF_WQT = OFF_XT + KC * BAND     # [128, 4, 512]
OFF_MISC = OFF_WQT + KC * C      # partition-0 row vectors:
#   bq row [1, 512] | bv row [1, 512] | ones [1, 128]
OFF_ID = OFF_MISC + 2 * C + 128  # [128, 128] identity
END_DMA1 = OFF_ID + 128
OFF_WKT = END_DMA1               # [128, 4, 512]
END_DMA2 = OFF_WKT + KC * C
OFF_CM = END_DMA2                # [128, 2, 256]
OFF_CMT = OFF_CM + JC * BAND
OFF_MW = OFF_CMT + JC * BAND     # [128, 2, 128]
END_DMA3 = OFF_MW + JC * RCH
OFF_WVT = END_DMA3               # [128, 4, 512]
FBLOB = OFF_WVT + KC * C

_CACHE = {}


def _slim_drain_and_barrier(self, tick_clock, wait_clock):
    """Cheaper TileContext exit: keep the global-clock drain (all work,
    including the output DMA, must complete) and the semaphore reset (NEFF
    re-executability), but use a sem-only barrier and drop the second
    all-engine barrier — saves ~6-8us of EVSEM butterfly per execution."""
    from concourse.vector_clock import ScopedClock

    drain_inst = self.nc.sync.drain()
    wait_clock.add_sem_waits(
        drain_inst.ins, ScopedClock({None: tick_clock.global_clock})
    )
    self.nc.all_engine_barrier(sem_only=True)
    popped = self.nc._tile_sem_poison_stack.pop()
    assert popped is self._sem_poison
    self.nc.clear_and_free_semaphores(list(self.sems.allocated().values()))


def _build_program():
    import concourse.bacc as bacc
    import concourse.mybir as mybir
    import concourse.tile as tile

    from concourse.tile_rust import add_dep_helper

    fp32 = mybir.dt.float32
    fp32r = mybir.dt.float32r
    nc = bacc.Bacc("TRN2", target_bir_lowering=False, debug=False)

    blob_d = nc.declare_dram_parameter("blob", [128, FBLOB], fp32, isOutput=False)
    out_d = nc.declare_dram_parameter("out", [RCH, C], fp32, isOutput=True)

    orig_dab = tile.TileContext._drain_and_barrier
    tile.TileContext._drain_and_barrier = _slim_drain_and_barrier
    try:
        _emit_body(nc, tile, mybir, add_dep_helper, blob_d, out_d, fp32, fp32r)
    finally:
        tile.TileContext._drain_and_barrier = orig_dab

    nc.compile()
    return nc


def _emit_body(nc, tile, mybir, add_dep_helper, blob_d, out_d, fp32, fp32r):
    with tile.TileContext(nc) as tc:
        with (
            tc.tile_pool(name="consts", bufs=1) as consts,
            tc.tile_pool(name="work", bufs=1) as work,
            tc.tile_pool(name="psum", bufs=1, space="PSUM") as psum,
        ):
            # blob is typed float32r so matmuls can consume it directly;
            # non-matmul users view it as fp32 (same bits).
            # Four priority-chained DMA pieces: compute on piece 1 starts
            # while pieces 2-4 stream in.
            blob = consts.tile([128, FBLOB], fp32r)
            blob_src = blob_d[:].bitcast(fp32r)
            bounds = [0, END_DMA1, END_DMA2, END_DMA3, FBLOB]
            prev = None
            for lo, hi in zip(bounds, bounds[1:]):
                d = nc.sync.dma_start(
                    out=blob[:, lo:hi], in_=blob_src[:, lo:hi]
                )
                if prev is not None:
                    add_dep_helper(d.ins, prev.ins, True,
                                   "input DMA priority chain")
                prev = d

            def seg(off, length):
                return blob[:, off:off + length]

            xt_sb = seg(OFF_XT, KC * BAND).rearrange("p (k j) -> p k j", k=KC)
            wqt_sb = seg(OFF_WQT, KC * C).rearrange("p (k j) -> p k j", k=KC)
            wkt_sb = seg(OFF_WKT, KC * C).rearrange("p (k j) -> p k j", k=KC)
            wvt_sb = seg(OFF_WVT, KC * C).rearrange("p (k j) -> p k j", k=KC)
            bqr_sb = blob[0:1, OFF_MISC:OFF_MISC + C]
            bvr_sb = blob[0:1, OFF_MISC + C:OFF_MISC + 2 * C]
            cm_sb = seg(OFF_CM, JC * BAND).rearrange("p (k t) -> p k t", k=JC)
            cmt_sb = seg(OFF_CMT, JC * BAND).rearrange("p (k j) -> p k j", k=JC)
            mw_sb = seg(OFF_MW, JC * RCH).rearrange("p (k r) -> p k r", k=JC)
            ident = seg(OFF_ID, 128)
            ones1 = blob[0:1, OFF_MISC + 2 * C:OFF_MISC + 2 * C + 128]

            # ---- projections ----
            # Q rows [r 128, c 512] (+bq via rank-1 ones matmul), then
            # transpose to QT chunks [c 128, r 128]
            ps_qrow = psum.tile([128, C], fp32, tag="ps_big", bufs=2)
            for k in range(KC):
                nc.tensor.matmul(
                    ps_qrow,
                    lhsT=xt_sb[:, k, 64:64 + RCH],
                    rhs=wqt_sb[:, k, :],
                    start=(k == 0),
                    stop=False,
                )
            nc.tensor.matmul(
                ps_qrow, lhsT=ones1, rhs=bqr_sb, start=False, stop=True,
            )
            q_sb = work.tile([128, C], fp32r)
            nc.vector.tensor_copy(out=q_sb, in_=ps_qrow)
            qt_sb = work.tile([128, KC, RCH], fp32r)
            for m in range(KC):
                ps_t = psum.tile([128, RCH], fp32r, tag="ps_t", bufs=1)
                nc.tensor.transpose(
                    ps_t, q_sb[:, m * 128:(m + 1) * 128], ident
                )
                nc.vector.tensor_copy(out=qt_sb[:, m, :], in_=ps_t)

            # KT[m][c_out 128, j 256]  (bk dropped: softmax-invariant)
            kt_sb = work.tile([128, KC, BAND], fp32r)
            for m in range(KC):
                ps_k = psum.tile([128, BAND], fp32, tag="ps_k", bufs=1)
                for k in range(KC):
                    nc.tensor.matmul(
                        ps_k,
                        lhsT=wkt_sb[:, k, m * 128:(m + 1) * 128],
                        rhs=xt_sb[:, k, :],
                        start=(k == 0),
                        stop=(k == KC - 1),
                    )
                nc.vector.tensor_copy(out=kt_sb[:, m, :], in_=ps_k)

            # V[jc][j 128, c 512] (+bv via rank-1 ones matmul)
            v_sb = work.tile([128, JC, C], fp32r)
            for jc in range(JC):
                ps_v = psum.tile([128, C], fp32, tag="ps_big", bufs=2)
                for k in range(KC):
                    nc.tensor.matmul(
                        ps_v,
                        lhsT=xt_sb[:, k, jc * 128:(jc + 1) * 128],
                        rhs=wvt_sb[:, k, :],
                        start=(k == 0),
                        stop=False,
                    )
                nc.tensor.matmul(
                    ps_v, lhsT=ones1, rhs=bvr_sb, start=False, stop=True,
                )
                nc.vector.tensor_copy(out=v_sb[:, jc, :], in_=ps_v)

            # ---- scores and softmax numerator ----
            ps_s = psum.tile([128, BAND], fp32, tag="ps_s", bufs=1)
            for k in range(KC):
                nc.tensor.matmul(
                    ps_s,
                    lhsT=qt_sb[:, k, :],
                    rhs=kt_sb[:, k, :],
                    start=(k == 0),
                    stop=(k == KC - 1),
                )
            negmax = work.tile([128, 1], fp32)
            nc.vector.reduce_max(
                negmax, ps_s, axis=mybir.AxisListType.X, negate=True
            )
            e_sb = work.tile([128, BAND], fp32r)
            nc.scalar.activation(
                out=e_sb, in_=ps_s,
                func=mybir.ActivationFunctionType.Exp,
                bias=negmax, scale=1.0,
            )

            # E^T chunks [j 128, r 128]
            et_sb = work.tile([128, JC, RCH], fp32r)
            for jc in range(JC):
                ps_t = psum.tile([128, RCH], fp32r, tag="ps_t", bufs=1)
                nc.tensor.transpose(
                    ps_t, e_sb[:, jc * 128:(jc + 1) * 128], ident
                )
                nc.vector.tensor_copy(out=et_sb[:, jc, :], in_=ps_t)

            # Z'[t 128, r 128] = sum_j Cm[j,t] E'[j,r];  W' = maskw / Z'
            w_sb = work.tile([128, JC, RCH], fp32r)
            for tch in range(JC):
                ps_z = psum.tile([128, RCH], fp32, tag="ps_z", bufs=1)
                for jc in range(JC):
                    nc.tensor.matmul(
                        ps_z,
                        lhsT=cm_sb[:, jc, tch * 128:(tch + 1) * 128],
                        rhs=et_sb[:, jc, :],
                        start=(jc == 0),
                        stop=(jc == JC - 1),
                    )
                rz = work.tile([128, RCH], fp32, tag="rz", bufs=2)
                nc.vector.reciprocal(out=rz, in_=ps_z)
                nc.vector.tensor_mul(
                    w_sb[:, tch, :], rz, mw_sb[:, tch, :].bitcast(fp32)
                )

            # U'[j 128, r 128] = sum_t Cm^T[t,j] W'[t,r];  A' = E' * U'
            a_sb = work.tile([128, JC, RCH], fp32r)
            for jc in range(JC):
                ps_u = psum.tile([128, RCH], fp32, tag="ps_u", bufs=1)
                for tch in range(JC):
                    nc.tensor.matmul(
                        ps_u,
                        lhsT=cmt_sb[:, tch, jc * 128:(jc + 1) * 128],
                        rhs=w_sb[:, tch, :],
                        start=(tch == 0),
                        stop=(tch == JC - 1),
                    )
                nc.vector.tensor_mul(
                    a_sb[:, jc, :], ps_u, et_sb[:, jc, :].bitcast(fp32)
                )

            # out rows [r 128, c 512] = sum_j A'[j,r]^T V[j,c]
            ps_o = psum.tile([128, C], fp32, tag="ps_o", bufs=1)
            for jc in range(JC):
                nc.tensor.matmul(
                    ps_o,
                    lhsT=a_sb[:, jc, :],
                    rhs=v_sb[:, jc, :],
                    start=(jc == 0),
                    stop=(jc == JC - 1),
                )
            o_sb = work.tile([128, C], fp32)
            nc.vector.tensor_copy(out=o_sb, in_=ps_o)
            nc.sync.dma_start(out=out_d[:], in_=o_sb)


def _pack128(arr):
    """[n*128, f] row-chunked -> [128, n*f] (chunk-major along free axis)."""
    n = arr.shape[0] // 128
    return np.ascontiguousarray(
        arr.reshape(n, 128, -1).transpose(1, 0, 2).reshape(128, -1)
    )


def _host_prep(image_features, Wq, bq, Wk, bk, Wv, bv, sample_idx):
    """Build the 8 per-core input blobs (pure index/layout work)."""
    x = np.asarray(image_features, np.float32)
    sample_idx = np.asarray(sample_idx)

    # per-tile multiplicities -> banded count matrix Cm[j, t] = m_t[j - t]
    mod = (sample_idx % W).astype(np.int64)                  # [T, S]
    m = np.zeros((T, W), np.float32)
    np.add.at(m, (np.arange(T)[:, None], mod), 1.0)
    m += 1.0
    Cm = np.zeros((N, N), np.float32)
    rows = np.arange(T)
    for w in range(W):
        Cm[rows + w, rows] = m[:, w]

    pos = np.arange(N)
    counts = (np.minimum(pos, N - W) - np.maximum(pos - W + 1, 0) + 1)

    # padded versions for uniform band slicing
    XTp = np.zeros((B, C, N + 2 * 64), np.float32)
    for b in range(B):
        XTp[b, :, 64:64 + N] = x[b].T
    Cmp = np.zeros((N + 2 * 64, N + 2 * 64), np.float32)
    Cmp[64:64 + N, 64:64 + N] = Cm

    wqt_p = _pack128(np.asarray(Wq, np.float32).T)
    wkt_p = _pack128(np.asarray(Wk, np.float32).T)
    wvt_p = _pack128(np.asarray(Wv, np.float32).T)

    in_maps = []
    for core in range(NCORES):
        b, rc = divmod(core, NCORES // B)
        r0 = rc * RCH
        xt = XTp[b, :, r0:r0 + BAND]
        cm = np.ascontiguousarray(Cmp[r0:r0 + BAND, r0:r0 + BAND])
        # all-zero columns (padded t) would give Z=0 -> 1/0*mask = NaN on
        # device; a diagonal 1 keeps Z finite there and is masked out of W
        zero_cols = ~cm.any(axis=0)
        cm[zero_cols, zero_cols] = 1.0
        tl = np.arange(BAND)
        rl = np.arange(RCH)
        tg = r0 - 64 + tl
        rg = r0 + rl
        d = rg[None, :] - tg[:, None]
        valid = (d >= 0) & (d <= W - 1) & (tg[:, None] >= 0) & (tg[:, None] <= T - 1)
        maskw = np.where(
            valid, 1.0 / counts[rg][None, :], 0.0
        ).astype(np.float32)

        blob = np.zeros((128, FBLOB), np.float32)
        blob[:, OFF_XT:OFF_XT + KC * BAND] = _pack128(xt)
        blob[:, OFF_WQT:OFF_WQT + KC * C] = wqt_p
        blob[:, OFF_WKT:OFF_WKT + KC * C] = wkt_p
        blob[:, OFF_WVT:OFF_WVT + KC * C] = wvt_p
        blob[0, OFF_MISC:OFF_MISC + C] = np.asarray(bq, np.float32)
        blob[0, OFF_MISC + C:OFF_MISC + 2 * C] = np.asarray(bv, np.float32)
        blob[0, OFF_MISC + 2 * C:OFF_MISC + 2 * C + 128] = 1.0
        blob[:, OFF_CM:OFF_CM + JC * BAND] = _pack128(cm)
        blob[:, OFF_CMT:OFF_CMT + JC * BAND] = _pack128(
            np.ascontiguousarray(cm.T)
        )
        blob[:, OFF_MW:OFF_MW + JC * RCH] = _pack128(maskw)
        blob[:, OFF_ID:OFF_ID + 128] = np.eye(128, dtype=np.float32)
        in_maps.append({"blob": blob})
    return in_maps


def run_on_cores(in_maps, trace=False, trace_cores=None):
    from concourse.bass_utils import run_bass_kernel_spmd

    if "nc" not in _CACHE:
        _CACHE["nc"] = _build_program()
    nc = _CACHE["nc"]
    return run_bass_kernel_spmd(
        nc, in_maps, list(range(NCORES)), trace=trace,
        trace_cores=(trace_cores or [0]) if trace else None,
    )


def kernel(image_features, Wq, bq, Wk, bk, Wv, bv, sample_idx):
    in_maps = _host_prep(image_features, Wq, bq, Wk, bk, Wv, bv, sample_idx)
    res = run_on_cores(in_maps, trace=False)
    out = np.empty((B, N, C), np.float32)
    for core in range(NCORES):
        b, rc = divmod(core, NCORES // B)
        out[b, rc * RCH:(rc + 1) * RCH, :] = res.results[core]["out"]
    return out


# revision 22
# speedup vs baseline: 1.4283x; 1.1005x over previous
"""Trainium2 Bass kernel for ConsistentSelfAttentionTile.

Reference semantics: T=449 overlapping 64-token tiles; each tile attends to
352 KV tokens = 288 sampled (from a 9x replication of the tile) + the tile
itself; outputs overlap-add, then divide by overlap counts.

Algebraic collapse used here (verified to ~1e-6 rel vs the jax reference):
  * rep[:, idx, :] == tile[:, idx % 64, :], so the sampled KV tokens are tile
    rows with integer multiplicities m_t[w] = 1 + #{s : idx[t,s] % 64 == w}.
  * Per-tile Q/K/V are slices of the full-sequence projections, so all
    per-tile 64x64 score blocks are diagonal blocks of one banded 512x512
    score matrix S = Q K^T (band |i-j| <= 63).
  * With E = exp(S - rowmax), Cm[j,t] = m_t[j-t] (banded), the full
    tile-softmax + overlap-add + count-divide collapses to
        Z = E @ Cm;  W = bandmask/(counts * Z);  U = W @ Cm^T;
        out = (E * U) @ V
    i.e. three extra banded 512x512 matmuls instead of 449 gathered
    attentions.
  * bk drops exactly: it shifts each row's scores by a constant, which the
    rowmax-subtracted softmax cancels bit-for-bit.

Sharding: 8 cores = 2 batches x 4 row-chunks of 128 output rows. Each core
computes its 128 rows end-to-end from a 256-column band of the input (no
cross-core communication); host slices/pads inputs and concatenates outputs.

Matmuls run in float32r (4x faster than fp32 at free-dim >= 256; ~13-bit
mantissa, measured 1.5e-4 rel err per 128-deep dot).

All per-core inputs are packed host-side into ONE [128, F] fp32 blob laid
out exactly as the SBUF destination -> a single input DMA (one DMA-queue
semaphore; instructions may carry only one sync wait).
"""

import os
import sys

import numpy as np

for _p in ("/opt/trn_rl_repo",):
    if _p not in sys.path and os.path.isdir(_p):
        sys.path.insert(0, _p)

B, N, C, W = 2, 512, 512, 64
T = N - W + 1          # 449 tiles
RCH = 128              # output rows per core
NCORES = 8
BAND = 256             # per-core j/t band width (columns [r0-64, r0+192))
KC = C // 128          # 4 contraction chunks
JC = BAND // 128       # 2 band chunks

# blob layout (fp32 elements per partition), ordered so the input DMA can be
# split into priority-chained pieces: [Q inputs | wkt | softmax consts | wvt]
OFF_XT = 0                       # [128, 4, 256]
OFF_WQT = OFF_XT + KC * BAND     # [128, 4, 512]
OFF_MISC = OFF_WQT + KC * C      # partition-0 row vectors:
#   bq row [1, 512] | bv row [1, 512] | ones [1, 128]
OFF_ID = OFF_MISC + 2 * C + 128  # [128, 128] identity
END_DMA1 = OFF_ID + 128
OFF_WKT = END_DMA1               # [128, 4, 512]
END_DMA2 = OFF_WKT + KC * C
OFF_CM = END_DMA2                # [128, 2, 256]
OFF_CMT = OFF_CM + JC * BAND
OFF_MW = OFF_CMT + JC * BAND     # [128, 2, 128]
END_DMA3 = OFF_MW + JC * RCH
OFF_WVT = END_DMA3               # [128, 4, 512]
FBLOB = OFF_WVT + KC * C

_CACHE = {}


def _slim_drain_and_barrier(self, tick_clock, wait_clock):
    """Cheaper TileContext exit. Every compute op in this kernel feeds the
    output DMA, so the final drain only needs to cover DMA-queue completion
    (not the full 27-proc global clock, whose multi-wait split costs an
    ~10us EVSEM butterfly). Engines are then synced with one sem-only
    barrier and the semaphores reset for NEFF re-executability."""
    from concourse.vector_clock import ScopedClock, VectorClock
    from concourse.tile_scheduler import dmasw_start_idx, N_PROCS

    g = tick_clock.global_clock
    dma_clock = VectorClock()
    for idx in range(dmasw_start_idx, N_PROCS):
        t = g.peek_next(idx) - 1
        if t > 0:
            dma_clock.require_at_least(idx, t)
    drain_inst = self.nc.sync.drain()
    wait_clock.add_sem_waits(drain_inst.ins, ScopedClock({None: dma_clock}))
    self.nc.all_engine_barrier(sem_only=True)
    popped = self.nc._tile_sem_poison_stack.pop()
    assert popped is self._sem_poison
    self.nc.clear_and_free_semaphores(list(self.sems.allocated().values()))


def _build_program():
    import concourse.bacc as bacc
    import concourse.mybir as mybir
    import concourse.tile as tile

    from concourse.tile_rust import add_dep_helper

    fp32 = mybir.dt.float32
    fp32r = mybir.dt.float32r
    # Bass's preamble ends with a full all-engine barrier (drains + EVSEM,
    # ~3us); a sem-only barrier gives the same cross-engine ordering for the
    # preamble const memsets at a fraction of the cost.
    orig_aeb = bacc.Bacc.all_engine_barrier

    def _sem_only_aeb(self, *, sem_only=False):
        return orig_aeb(self, sem_only=True)

    bacc.Bacc.all_engine_barrier = _sem_only_aeb
    try:
        nc = bacc.Bacc("TRN2", target_bir_lowering=False, debug=False)
    finally:
        bacc.Bacc.all_engine_barrier = orig_aeb

    blob_d = nc.declare_dram_parameter("blob", [128, FBLOB], fp32, isOutput=False)
    out_d = nc.declare_dram_parameter("out", [RCH, C], fp32, isOutput=True)

    orig_dab = tile.TileContext._drain_and_barrier
    tile.TileContext._drain_and_barrier = _slim_drain_and_barrier
    try:
        _emit_body(nc, tile, mybir, add_dep_helper, blob_d, out_d, fp32, fp32r)
    finally:
        tile.TileContext._drain_and_barrier = orig_dab

    nc.compile()
    return nc


def _emit_body(nc, tile, mybir, add_dep_helper, blob_d, out_d, fp32, fp32r):
    with tile.TileContext(nc) as tc:
        with (
            tc.tile_pool(name="consts", bufs=1) as consts,
            tc.tile_pool(name="work", bufs=1) as work,
            tc.tile_pool(name="psum", bufs=1, space="PSUM") as psum,
        ):
            # blob is typed float32r so matmuls can consume it directly;
            # non-matmul users view it as fp32 (same bits).
            # Four priority-chained DMA pieces: compute on piece 1 starts
            # while pieces 2-4 stream in.
            blob = consts.tile([128, FBLOB], fp32r)
            blob_src = blob_d[:].bitcast(fp32r)
            # Groups are priority-chained (group g+1 starts after group g);
            # within a group, two pieces run on parallel DMA queues since a
            # single HWDGE queue tops out around ~200 GB/s.
            bounds = [0, END_DMA1, END_DMA2, END_DMA3, FBLOB]
            prev_group = []
            for lo, hi in zip(bounds, bounds[1:]):
                mid = (lo + hi) // 2
                cur_group = []
                for a, b in ((lo, mid), (mid, hi)):
                    d = nc.sync.dma_start(
                        out=blob[:, a:b], in_=blob_src[:, a:b]
                    )
                    for p in prev_group:
                        add_dep_helper(d.ins, p.ins, True,
                                       "input DMA priority chain")
                    cur_group.append(d)
                prev_group = cur_group

            def seg(off, length):
                return blob[:, off:off + length]

            xt_sb = seg(OFF_XT, KC * BAND).rearrange("p (k j) -> p k j", k=KC)
            wqt_sb = seg(OFF_WQT, KC * C).rearrange("p (k j) -> p k j", k=KC)
            wkt_sb = seg(OFF_WKT, KC * C).rearrange("p (k j) -> p k j", k=KC)
            wvt_sb = seg(OFF_WVT, KC * C).rearrange("p (k j) -> p k j", k=KC)
            bqr_sb = blob[0:1, OFF_MISC:OFF_MISC + C]
            bvr_sb = blob[0:1, OFF_MISC + C:OFF_MISC + 2 * C]
            cm_sb = seg(OFF_CM, JC * BAND).rearrange("p (k t) -> p k t", k=JC)
            cmt_sb = seg(OFF_CMT, JC * BAND).rearrange("p (k j) -> p k j", k=JC)
            mw_sb = seg(OFF_MW, JC * RCH).rearrange("p (k r) -> p k r", k=JC)
            ident = seg(OFF_ID, 128)
            ones1 = blob[0:1, OFF_MISC + 2 * C:OFF_MISC + 2 * C + 128]

            # ---- projections ----
            # Q rows [r 128, c 512] (+bq via rank-1 ones matmul), then
            # transpose to QT chunks [c 128, r 128]
            ps_qrow = psum.tile([128, C], fp32, tag="ps_big", bufs=2)
            for k in range(KC):
                nc.tensor.matmul(
                    ps_qrow,
                    lhsT=xt_sb[:, k, 64:64 + RCH],
                    rhs=wqt_sb[:, k, :],
                    start=(k == 0),
                    stop=False,
                )
            nc.tensor.matmul(
                ps_qrow, lhsT=ones1, rhs=bqr_sb, start=False, stop=True,
            )
            q_sb = work.tile([128, C], fp32r)
            nc.vector.tensor_copy(out=q_sb, in_=ps_qrow)
            qt_sb = work.tile([128, KC, RCH], fp32r)
            for m in range(KC):
                ps_t = psum.tile([128, RCH], fp32r, tag="ps_t", bufs=1)
                nc.tensor.transpose(
                    ps_t, q_sb[:, m * 128:(m + 1) * 128], ident
                )
                nc.vector.tensor_copy(out=qt_sb[:, m, :], in_=ps_t)

            # KT[m][c_out 128, j 256]  (bk dropped: softmax-invariant)
            kt_sb = work.tile([128, KC, BAND], fp32r)
            for m in range(KC):
                ps_k = psum.tile([128, BAND], fp32, tag="ps_k", bufs=1)
                for k in range(KC):
                    nc.tensor.matmul(
                        ps_k,
                        lhsT=wkt_sb[:, k, m * 128:(m + 1) * 128],
                        rhs=xt_sb[:, k, :],
                        start=(k == 0),
                        stop=(k == KC - 1),
                    )
                nc.vector.tensor_copy(out=kt_sb[:, m, :], in_=ps_k)

            # V[jc][j 128, c 512] (+bv via rank-1 ones matmul)
            v_sb = work.tile([128, JC, C], fp32r)
            for jc in range(JC):
                ps_v = psum.tile([128, C], fp32, tag="ps_big", bufs=2)
                for k in range(KC):
                    nc.tensor.matmul(
                        ps_v,
                        lhsT=xt_sb[:, k, jc * 128:(jc + 1) * 128],
                        rhs=wvt_sb[:, k, :],
                        start=(k == 0),
                        stop=False,
                    )
                nc.tensor.matmul(
                    ps_v, lhsT=ones1, rhs=bvr_sb, start=False, stop=True,
                )
                nc.vector.tensor_copy(out=v_sb[:, jc, :], in_=ps_v)

            # ---- scores and softmax numerator ----
            ps_s = psum.tile([128, BAND], fp32, tag="ps_s", bufs=1)
            for k in range(KC):
                nc.tensor.matmul(
                    ps_s,
                    lhsT=qt_sb[:, k, :],
                    rhs=kt_sb[:, k, :],
                    start=(k == 0),
                    stop=(k == KC - 1),
                )
            negmax = work.tile([128, 1], fp32)
            nc.vector.reduce_max(
                negmax, ps_s, axis=mybir.AxisListType.X, negate=True
            )
            e_sb = work.tile([128, BAND], fp32r)
            nc.scalar.activation(
                out=e_sb, in_=ps_s,
                func=mybir.ActivationFunctionType.Exp,
                bias=negmax, scale=1.0,
            )

            # E^T chunks [j 128, r 128]
            et_sb = work.tile([128, JC, RCH], fp32r)
            for jc in range(JC):
                ps_t = psum.tile([128, RCH], fp32r, tag="ps_t", bufs=1)
                nc.tensor.transpose(
                    ps_t, e_sb[:, jc * 128:(jc + 1) * 128], ident
                )
                nc.vector.tensor_copy(out=et_sb[:, jc, :], in_=ps_t)

            # Z'[t 128, r 128] = sum_j Cm[j,t] E'[j,r];  W' = maskw / Z'
            w_sb = work.tile([128, JC, RCH], fp32r)
            for tch in range(JC):
                ps_z = psum.tile([128, RCH], fp32, tag="ps_z", bufs=1)
                for jc in range(JC):
                    nc.tensor.matmul(
                        ps_z,
                        lhsT=cm_sb[:, jc, tch * 128:(tch + 1) * 128],
                        rhs=et_sb[:, jc, :],
                        start=(jc == 0),
                        stop=(jc == JC - 1),
                    )
                rz = work.tile([128, RCH], fp32, tag="rz", bufs=2)
                nc.vector.reciprocal(out=rz, in_=ps_z)
                nc.vector.tensor_mul(
                    w_sb[:, tch, :], rz, mw_sb[:, tch, :].bitcast(fp32)
                )

            # U'[j 128, r 128] = sum_t Cm^T[t,j] W'[t,r];  A' = E' * U'
            a_sb = work.tile([128, JC, RCH], fp32r)
            for jc in range(JC):
                ps_u = psum.tile([128, RCH], fp32, tag="ps_u", bufs=1)
                for tch in range(JC):
                    nc.tensor.matmul(
                        ps_u,
                        lhsT=cmt_sb[:, tch, jc * 128:(jc + 1) * 128],
                        rhs=w_sb[:, tch, :],
                        start=(tch == 0),
                        stop=(tch == JC - 1),
                    )
                nc.vector.tensor_mul(
                    a_sb[:, jc, :], ps_u, et_sb[:, jc, :].bitcast(fp32)
                )

            # out rows [r 128, c 512] = sum_j A'[j,r]^T V[j,c]
            ps_o = psum.tile([128, C], fp32, tag="ps_o", bufs=1)
            for jc in range(JC):
                nc.tensor.matmul(
                    ps_o,
                    lhsT=a_sb[:, jc, :],
                    rhs=v_sb[:, jc, :],
                    start=(jc == 0),
                    stop=(jc == JC - 1),
                )
            o_sb = work.tile([128, C], fp32)
            nc.vector.tensor_copy(out=o_sb, in_=ps_o)
            nc.sync.dma_start(out=out_d[:], in_=o_sb)


def _pack128(arr):
    """[n*128, f] row-chunked -> [128, n*f] (chunk-major along free axis)."""
    n = arr.shape[0] // 128
    return np.ascontiguousarray(
        arr.reshape(n, 128, -1).transpose(1, 0, 2).reshape(128, -1)
    )


def _host_prep(image_features, Wq, bq, Wk, bk, Wv, bv, sample_idx):
    """Build the 8 per-core input blobs (pure index/layout work)."""
    x = np.asarray(image_features, np.float32)
    sample_idx = np.asarray(sample_idx)

    # per-tile multiplicities -> banded count matrix Cm[j, t] = m_t[j - t]
    mod = (sample_idx % W).astype(np.int64)                  # [T, S]
    m = np.zeros((T, W), np.float32)
    np.add.at(m, (np.arange(T)[:, None], mod), 1.0)
    m += 1.0
    Cm = np.zeros((N, N), np.float32)
    rows = np.arange(T)
    for w in range(W):
        Cm[rows + w, rows] = m[:, w]

    pos = np.arange(N)
    counts = (np.minimum(pos, N - W) - np.maximum(pos - W + 1, 0) + 1)

    # padded versions for uniform band slicing
    XTp = np.zeros((B, C, N + 2 * 64), np.float32)
    for b in range(B):
        XTp[b, :, 64:64 + N] = x[b].T
    Cmp = np.zeros((N + 2 * 64, N + 2 * 64), np.float32)
    Cmp[64:64 + N, 64:64 + N] = Cm

    wqt_p = _pack128(np.asarray(Wq, np.float32).T)
    wkt_p = _pack128(np.asarray(Wk, np.float32).T)
    wvt_p = _pack128(np.asarray(Wv, np.float32).T)

    in_maps = []
    for core in range(NCORES):
        b, rc = divmod(core, NCORES // B)
        r0 = rc * RCH
        xt = XTp[b, :, r0:r0 + BAND]
        cm = np.ascontiguousarray(Cmp[r0:r0 + BAND, r0:r0 + BAND])
        # all-zero columns (padded t) would give Z=0 -> 1/0*mask = NaN on
        # device; a diagonal 1 keeps Z finite there and is masked out of W
        zero_cols = ~cm.any(axis=0)
        cm[zero_cols, zero_cols] = 1.0
        tl = np.arange(BAND)
        rl = np.arange(RCH)
        tg = r0 - 64 + tl
        rg = r0 + rl
        d = rg[None, :] - tg[:, None]
        valid = (d >= 0) & (d <= W - 1) & (tg[:, None] >= 0) & (tg[:, None] <= T - 1)
        maskw = np.where(
            valid, 1.0 / counts[rg][None, :], 0.0
        ).astype(np.float32)

        blob = np.zeros((128, FBLOB), np.float32)
        blob[:, OFF_XT:OFF_XT + KC * BAND] = _pack128(xt)
        blob[:, OFF_WQT:OFF_WQT + KC * C] = wqt_p
        blob[:, OFF_WKT:OFF_WKT + KC * C] = wkt_p
        blob[:, OFF_WVT:OFF_WVT + KC * C] = wvt_p
        blob[0, OFF_MISC:OFF_MISC + C] = np.asarray(bq, np.float32)
        blob[0, OFF_MISC + C:OFF_MISC + 2 * C] = np.asarray(bv, np.float32)
        blob[0, OFF_MISC + 2 * C:OFF_MISC + 2 * C + 128] = 1.0
        blob[:, OFF_CM:OFF_CM + JC * BAND] = _pack128(cm)
        blob[:, OFF_CMT:OFF_CMT + JC * BAND] = _pack128(
            np.ascontiguousarray(cm.T)
        )
        blob[:, OFF_MW:OFF_MW + JC * RCH] = _pack128(maskw)
        blob[:, OFF_ID:OFF_ID + 128] = np.eye(128, dtype=np.float32)
        in_maps.append({"blob": blob})
    return in_maps


def run_on_cores(in_maps, trace=False, trace_cores=None):
    from concourse.bass_utils import run_bass_kernel_spmd

    if "nc" not in _CACHE:
        _CACHE["nc"] = _build_program()
    nc = _CACHE["nc"]
    return run_bass_kernel_spmd(
        nc, in_maps, list(range(NCORES)), trace=trace,
        trace_cores=(trace_cores or [0]) if trace else None,
    )


def kernel(image_features, Wq, bq, Wk, bk, Wv, bv, sample_idx):
    in_maps = _host_prep(image_features, Wq, bq, Wk, bk, Wv, bv, sample_idx)
    res = run_on_cores(in_maps, trace=False)
    out = np.empty((B, N, C), np.float32)
    for core in range(NCORES):
        b, rc = divmod(core, NCORES // B)
        out[b, rc * RCH:(rc + 1) * RCH, :] = res.results[core]["out"]
    return out


# revision 23
# speedup vs baseline: 1.5281x; 1.0699x over previous
"""Trainium2 Bass kernel for ConsistentSelfAttentionTile.

Reference semantics: T=449 overlapping 64-token tiles; each tile attends to
352 KV tokens = 288 sampled (from a 9x replication of the tile) + the tile
itself; outputs overlap-add, then divide by overlap counts.

Algebraic collapse used here (verified to ~1e-6 rel vs the jax reference):
  * rep[:, idx, :] == tile[:, idx % 64, :], so the sampled KV tokens are tile
    rows with integer multiplicities m_t[w] = 1 + #{s : idx[t,s] % 64 == w}.
  * Per-tile Q/K/V are slices of the full-sequence projections, so all
    per-tile 64x64 score blocks are diagonal blocks of one banded 512x512
    score matrix S = Q K^T (band |i-j| <= 63).
  * With E = exp(S - rowmax), Cm[j,t] = m_t[j-t] (banded), the full
    tile-softmax + overlap-add + count-divide collapses to
        Z = E @ Cm;  W = bandmask/(counts * Z);  U = W @ Cm^T;
        out = (E * U) @ V
    i.e. three extra banded 512x512 matmuls instead of 449 gathered
    attentions.
  * bk drops exactly: it shifts each row's scores by a constant, which the
    rowmax-subtracted softmax cancels bit-for-bit.

Sharding: 8 cores = 2 batches x 4 row-chunks of 128 output rows. Each core
computes its 128 rows end-to-end from a 256-column band of the input (no
cross-core communication); host slices/pads inputs and concatenates outputs.

Precision plan: x and the three weight matrices ship as fp16 (halves the
DMA, which is the bottleneck at ~210 GB/s/core); all matmul products
accumulate in fp32 PSUM. The score/softmax chain (Q^T, K^T, S, E, Cm, W, U)
stays in float32r (~13-bit mantissa; fp16 E would underflow to subnormals
whenever a row's in-band max sits ~16 below its window max). The value path
(V, A, out-matmul) is fp16, where rounding only mixes linearly.

Per-core inputs are packed host-side into two blobs laid out exactly as
their SBUF destinations, DMA'd in priority-chained groups (2 parallel
queues per group) so compute starts after the first ~1 MB.
"""

import os
import sys

import numpy as np

for _p in ("/opt/trn_rl_repo",):
    if _p not in sys.path and os.path.isdir(_p):
        sys.path.insert(0, _p)

B, N, C, W = 2, 512, 512, 64
T = N - W + 1          # 449 tiles
RCH = 128              # output rows per core
NCORES = 8
BAND = 256             # per-core j/t band width (columns [r0-64, r0+192))
KC = C // 128          # 4 contraction chunks
JC = BAND // 128       # 2 band chunks

# blob16 layout (fp16 elements per partition)
OFF_XT = 0                       # [128, 4, 256]
OFF_WQT = OFF_XT + KC * BAND     # [128, 4, 512]
OFF_MISC = OFF_WQT + KC * C      # p0 rows: bq [512] | bv [512] | ones [128]
END16_G1 = OFF_MISC + 2 * C + 128
OFF_WKT = END16_G1               # [128, 4, 512]
END16_G2 = OFF_WKT + KC * C
OFF_WVT = END16_G2               # [128, 4, 512]
F16 = OFF_WVT + KC * C

# blob32 layout (fp32 elements per partition)
OFF_CM = 0                       # [128, 2, 256]
OFF_CMT = OFF_CM + JC * BAND
OFF_MW = OFF_CMT + JC * BAND     # [128, 2, 128]
OFF_ID = OFF_MW + JC * RCH       # [128, 128]
F32 = OFF_ID + 128

_CACHE = {}


def _slim_drain_and_barrier(self, tick_clock, wait_clock):
    """Cheaper TileContext exit. Every compute op in this kernel feeds the
    output DMA, so the final drain only needs to cover DMA-queue completion
    (not the full 27-proc global clock, whose multi-wait split costs an
    ~10us EVSEM butterfly). Engines are then synced with one sem-only
    barrier and the semaphores reset for NEFF re-executability."""
    from concourse.vector_clock import ScopedClock, VectorClock
    from concourse.tile_scheduler import dmasw_start_idx, N_PROCS

    g = tick_clock.global_clock
    dma_clock = VectorClock()
    for idx in range(dmasw_start_idx, N_PROCS):
        t = g.peek_next(idx) - 1
        if t > 0:
            dma_clock.require_at_least(idx, t)
    drain_inst = self.nc.sync.drain()
    wait_clock.add_sem_waits(drain_inst.ins, ScopedClock({None: dma_clock}))
    self.nc.all_engine_barrier(sem_only=True)
    popped = self.nc._tile_sem_poison_stack.pop()
    assert popped is self._sem_poison
    self.nc.clear_and_free_semaphores(list(self.sems.allocated().values()))


def _build_program():
    import concourse.bacc as bacc
    import concourse.mybir as mybir
    import concourse.tile as tile

    fp32 = mybir.dt.float32
    fp16 = mybir.dt.float16
    # Bass's preamble ends with a full all-engine barrier (drains + EVSEM,
    # ~3us); a sem-only barrier gives the same cross-engine ordering for the
    # preamble const memsets at a fraction of the cost.
    orig_aeb = bacc.Bacc.all_engine_barrier

    def _sem_only_aeb(self, *, sem_only=False):
        return orig_aeb(self, sem_only=True)

    bacc.Bacc.all_engine_barrier = _sem_only_aeb
    try:
        nc = bacc.Bacc("TRN2", target_bir_lowering=False, debug=False)
    finally:
        bacc.Bacc.all_engine_barrier = orig_aeb

    b16_d = nc.declare_dram_parameter("blob16", [128, F16], fp16, isOutput=False)
    b32_d = nc.declare_dram_parameter("blob32", [128, F32], fp32, isOutput=False)
    out_d = nc.declare_dram_parameter("out", [RCH, C], fp32, isOutput=True)

    orig_dab = tile.TileContext._drain_and_barrier
    tile.TileContext._drain_and_barrier = _slim_drain_and_barrier
    try:
        _emit_body(nc, tile, mybir, b16_d, b32_d, out_d)
    finally:
        tile.TileContext._drain_and_barrier = orig_dab

    nc.compile()
    return nc


def _emit_body(nc, tile, mybir, b16_d, b32_d, out_d):
    from concourse.tile_rust import add_dep_helper

    fp32 = mybir.dt.float32
    fp32r = mybir.dt.float32r
    fp16 = mybir.dt.float16

    with tile.TileContext(nc) as tc:
        with (
            tc.tile_pool(name="consts", bufs=1) as consts,
            tc.tile_pool(name="work", bufs=1) as work,
            tc.tile_pool(name="psum", bufs=1, space="PSUM") as psum,
        ):
            b16 = consts.tile([128, F16], fp16)
            b32 = consts.tile([128, F32], fp32r)
            b32_src = b32_d[:].bitcast(fp32r)
            # Priority-chained DMA groups; 2 parallel queues per group
            # (a single HWDGE queue tops out around ~200 GB/s).
            groups = [
                (b16, b16_d[:], 0, END16_G1),
                (b16, b16_d[:], END16_G1, END16_G2),
                (b32, b32_src, 0, F32),
                (b16, b16_d[:], END16_G2, F16),
            ]
            prev_group = []
            for dst, src, lo, hi in groups:
                mid = (lo + hi) // 2
                cur_group = []
                for a, b in ((lo, mid), (mid, hi)):
                    d = nc.sync.dma_start(out=dst[:, a:b], in_=src[:, a:b])
                    for p in prev_group:
                        add_dep_helper(d.ins, p.ins, True,
                                       "input DMA priority chain")
                    cur_group.append(d)
                prev_group = cur_group

            xt_sb = b16[:, OFF_XT:OFF_XT + KC * BAND].rearrange(
                "p (k j) -> p k j", k=KC)
            wqt_sb = b16[:, OFF_WQT:OFF_WQT + KC * C].rearrange(
                "p (k j) -> p k j", k=KC)
            wkt_sb = b16[:, OFF_WKT:OFF_WKT + KC * C].rearrange(
                "p (k j) -> p k j", k=KC)
            wvt_sb = b16[:, OFF_WVT:OFF_WVT + KC * C].rearrange(
                "p (k j) -> p k j", k=KC)
            bqr_sb = b16[0:1, OFF_MISC:OFF_MISC + C]
            bvr_sb = b16[0:1, OFF_MISC + C:OFF_MISC + 2 * C]
            ones1 = b16[0:1, OFF_MISC + 2 * C:OFF_MISC + 2 * C + 128]
            cm_sb = b32[:, OFF_CM:OFF_CM + JC * BAND].rearrange(
                "p (k t) -> p k t", k=JC)
            cmt_sb = b32[:, OFF_CMT:OFF_CMT + JC * BAND].rearrange(
                "p (k j) -> p k j", k=JC)
            mw_sb = b32[:, OFF_MW:OFF_MW + JC * RCH].rearrange(
                "p (k r) -> p k r", k=JC)
            ident = b32[:, OFF_ID:OFF_ID + 128]

            # ---- projections (fp16 inputs, fp32 PSUM accumulation) ----
            # Q rows [r 128, c 512] (+bq via rank-1 ones matmul), then
            # transpose to QT chunks [c 128, r 128] in fp32r
            ps_qrow = psum.tile([128, C], fp32, tag="ps_big", bufs=2)
            for k in range(KC):
                nc.tensor.matmul(
                    ps_qrow,
                    lhsT=xt_sb[:, k, 64:64 + RCH],
                    rhs=wqt_sb[:, k, :],
                    start=(k == 0),
                    stop=False,
                )
            nc.tensor.matmul(
                ps_qrow, lhsT=ones1, rhs=bqr_sb, start=False, stop=True,
            )
            q_sb = work.tile([128, C], fp32r)
            nc.vector.tensor_copy(out=q_sb, in_=ps_qrow)
            qt_sb = work.tile([128, KC, RCH], fp32r)
            for m in range(KC):
                ps_t = psum.tile([128, RCH], fp32r, tag="ps_t", bufs=1)
                nc.tensor.transpose(
                    ps_t, q_sb[:, m * 128:(m + 1) * 128], ident
                )
                nc.vector.tensor_copy(out=qt_sb[:, m, :], in_=ps_t)

            # KT[m][c_out 128, j 256]  (bk dropped: softmax-invariant)
            kt_sb = work.tile([128, KC, BAND], fp32r)
            for m in range(KC):
                ps_k = psum.tile([128, BAND], fp32, tag="ps_k", bufs=1)
                for k in range(KC):
                    nc.tensor.matmul(
                        ps_k,
                        lhsT=wkt_sb[:, k, m * 128:(m + 1) * 128],
                        rhs=xt_sb[:, k, :],
                        start=(k == 0),
                        stop=(k == KC - 1),
                    )
                nc.vector.tensor_copy(out=kt_sb[:, m, :], in_=ps_k)

            # V[jc][j 128, c 512] (+bv via rank-1 ones matmul), fp16
            v_sb = work.tile([128, JC, C], fp16)
            for jc in range(JC):
                ps_v = psum.tile([128, C], fp32, tag="ps_big", bufs=2)
                for k in range(KC):
                    nc.tensor.matmul(
                        ps_v,
                        lhsT=xt_sb[:, k, jc * 128:(jc + 1) * 128],
                        rhs=wvt_sb[:, k, :],
                        start=(k == 0),
                        stop=False,
                    )
                nc.tensor.matmul(
                    ps_v, lhsT=ones1, rhs=bvr_sb, start=False, stop=True,
                )
                nc.vector.tensor_copy(out=v_sb[:, jc, :], in_=ps_v)

            # ---- scores and softmax numerator (fp32r) ----
            ps_s = psum.tile([128, BAND], fp32, tag="ps_s", bufs=1)
            for k in range(KC):
                nc.tensor.matmul(
                    ps_s,
                    lhsT=qt_sb[:, k, :],
                    rhs=kt_sb[:, k, :],
                    start=(k == 0),
                    stop=(k == KC - 1),
                )
            negmax = work.tile([128, 1], fp32)
            nc.vector.reduce_max(
                negmax, ps_s, axis=mybir.AxisListType.X, negate=True
            )
            e_sb = work.tile([128, BAND], fp32r)
            nc.scalar.activation(
                out=e_sb, in_=ps_s,
                func=mybir.ActivationFunctionType.Exp,
                bias=negmax, scale=1.0,
            )

            # E^T chunks [j 128, r 128]
            et_sb = work.tile([128, JC, RCH], fp32r)
            for jc in range(JC):
                ps_t = psum.tile([128, RCH], fp32r, tag="ps_t", bufs=1)
                nc.tensor.transpose(
                    ps_t, e_sb[:, jc * 128:(jc + 1) * 128], ident
                )
                nc.vector.tensor_copy(out=et_sb[:, jc, :], in_=ps_t)

            # Z'[t 128, r 128] = sum_j Cm[j,t] E'[j,r];  W' = maskw / Z'
            w_sb = work.tile([128, JC, RCH], fp32r)
            for tch in range(JC):
                ps_z = psum.tile([128, RCH], fp32, tag="ps_z", bufs=1)
                for jc in range(JC):
                    nc.tensor.matmul(
                        ps_z,
                        lhsT=cm_sb[:, jc, tch * 128:(tch + 1) * 128],
                        rhs=et_sb[:, jc, :],
                        start=(jc == 0),
                        stop=(jc == JC - 1),
                    )
                rz = work.tile([128, RCH], fp32, tag="rz", bufs=2)
                nc.vector.reciprocal(out=rz, in_=ps_z)
                nc.vector.tensor_mul(
                    w_sb[:, tch, :], rz, mw_sb[:, tch, :].bitcast(fp32)
                )

            # U'[j 128, r 128] = sum_t Cm^T[t,j] W'[t,r];  A' = E' * U'
            a_sb = work.tile([128, JC, RCH], fp16)
            for jc in range(JC):
                ps_u = psum.tile([128, RCH], fp32, tag="ps_u", bufs=1)
                for tch in range(JC):
                    nc.tensor.matmul(
                        ps_u,
                        lhsT=cmt_sb[:, tch, jc * 128:(jc + 1) * 128],
                        rhs=w_sb[:, tch, :],
                        start=(tch == 0),
                        stop=(tch == JC - 1),
                    )
                nc.vector.tensor_mul(
                    a_sb[:, jc, :], ps_u, et_sb[:, jc, :].bitcast(fp32)
                )

            # out rows [r 128, c 512] = sum_j A'[j,r]^T V[j,c]  (fp16)
            ps_o = psum.tile([128, C], fp32, tag="ps_o", bufs=1)
            for jc in range(JC):
                nc.tensor.matmul(
                    ps_o,
                    lhsT=a_sb[:, jc, :],
                    rhs=v_sb[:, jc, :],
                    start=(jc == 0),
                    stop=(jc == JC - 1),
                )
            o_sb = work.tile([128, C], fp32)
            nc.vector.tensor_copy(out=o_sb, in_=ps_o)
            nc.sync.dma_start(out=out_d[:], in_=o_sb)


def _pack128(arr):
    """[n*128, f] row-chunked -> [128, n*f] (chunk-major along free axis)."""
    n = arr.shape[0] // 128
    return np.ascontiguousarray(
        arr.reshape(n, 128, -1).transpose(1, 0, 2).reshape(128, -1)
    )


def _host_prep(image_features, Wq, bq, Wk, bk, Wv, bv, sample_idx):
    """Build the 8 per-core input blobs (pure index/layout work)."""
    x = np.asarray(image_features, np.float32)
    sample_idx = np.asarray(sample_idx)

    # per-tile multiplicities -> banded count matrix Cm[j, t] = m_t[j - t]
    mod = (sample_idx % W).astype(np.int64)                  # [T, S]
    m = np.zeros((T, W), np.float32)
    np.add.at(m, (np.arange(T)[:, None], mod), 1.0)
    m += 1.0
    Cm = np.zeros((N, N), np.float32)
    rows = np.arange(T)
    for w in range(W):
        Cm[rows + w, rows] = m[:, w]

    pos = np.arange(N)
    counts = (np.minimum(pos, N - W) - np.maximum(pos - W + 1, 0) + 1)

    # padded versions for uniform band slicing
    XTp = np.zeros((B, C, N + 2 * 64), np.float16)
    for b in range(B):
        XTp[b, :, 64:64 + N] = x[b].T.astype(np.float16)
    Cmp = np.zeros((N + 2 * 64, N + 2 * 64), np.float32)
    Cmp[64:64 + N, 64:64 + N] = Cm

    wqt_p = _pack128(np.asarray(Wq, np.float32).T.astype(np.float16))
    wkt_p = _pack128(np.asarray(Wk, np.float32).T.astype(np.float16))
    wvt_p = _pack128(np.asarray(Wv, np.float32).T.astype(np.float16))

    in_maps = []
    for core in range(NCORES):
        b, rc = divmod(core, NCORES // B)
        r0 = rc * RCH
        xt = XTp[b, :, r0:r0 + BAND]
        cm = np.ascontiguousarray(Cmp[r0:r0 + BAND, r0:r0 + BAND])
        # all-zero columns (padded t) would give Z=0 -> 1/0*mask = NaN on
        # device; a diagonal 1 keeps Z finite there and is masked out of W
        zero_cols = ~cm.any(axis=0)
        cm[zero_cols, zero_cols] = 1.0
        tl = np.arange(BAND)
        rl = np.arange(RCH)
        tg = r0 - 64 + tl
        rg = r0 + rl
        d = rg[None, :] - tg[:, None]
        valid = (d >= 0) & (d <= W - 1) & (tg[:, None] >= 0) & (tg[:, None] <= T - 1)
        maskw = np.where(
            valid, 1.0 / counts[rg][None, :], 0.0
        ).astype(np.float32)

        b16 = np.zeros((128, F16), np.float16)
        b16[:, OFF_XT:OFF_XT + KC * BAND] = _pack128(xt)
        b16[:, OFF_WQT:OFF_WQT + KC * C] = wqt_p
        b16[:, OFF_WKT:OFF_WKT + KC * C] = wkt_p
        b16[:, OFF_WVT:OFF_WVT + KC * C] = wvt_p
        b16[0, OFF_MISC:OFF_MISC + C] = np.asarray(bq, np.float32)
        b16[0, OFF_MISC + C:OFF_MISC + 2 * C] = np.asarray(bv, np.float32)
        b16[0, OFF_MISC + 2 * C:OFF_MISC + 2 * C + 128] = 1.0

        b32 = np.zeros((128, F32), np.float32)
        b32[:, OFF_CM:OFF_CM + JC * BAND] = _pack128(cm)
        b32[:, OFF_CMT:OFF_CMT + JC * BAND] = _pack128(
            np.ascontiguousarray(cm.T)
        )
        b32[:, OFF_MW:OFF_MW + JC * RCH] = _pack128(maskw)
        b32[:, OFF_ID:OFF_ID + 128] = np.eye(128, dtype=np.float32)
        in_maps.append({"blob16": b16, "blob32": b32})
    return in_maps


def run_on_cores(in_maps, trace=False, trace_cores=None):
    from concourse.bass_utils import run_bass_kernel_spmd

    if "nc" not in _CACHE:
        _CACHE["nc"] = _build_program()
    nc = _CACHE["nc"]
    return run_bass_kernel_spmd(
        nc, in_maps, list(range(NCORES)), trace=trace,
        trace_cores=(trace_cores or [0]) if trace else None,
    )


def kernel(image_features, Wq, bq, Wk, bk, Wv, bv, sample_idx):
    in_maps = _host_prep(image_features, Wq, bq, Wk, bk, Wv, bv, sample_idx)
    res = run_on_cores(in_maps, trace=False)
    out = np.empty((B, N, C), np.float32)
    for core in range(NCORES):
        b, rc = divmod(core, NCORES // B)
        out[b, rc * RCH:(rc + 1) * RCH, :] = res.results[core]["out"]
    return out


# revision 24
# speedup vs baseline: 1.6123x; 1.0551x over previous
"""Trainium2 Bass kernel for ConsistentSelfAttentionTile.

Reference semantics: T=449 overlapping 64-token tiles; each tile attends to
352 KV tokens = 288 sampled (from a 9x replication of the tile) + the tile
itself; outputs overlap-add, then divide by overlap counts.

Algebraic collapse used here (verified to ~1e-6 rel vs the jax reference):
  * rep[:, idx, :] == tile[:, idx % 64, :], so the sampled KV tokens are tile
    rows with integer multiplicities m_t[w] = 1 + #{s : idx[t,s] % 64 == w}.
  * Per-tile Q/K/V are slices of the full-sequence projections, so all
    per-tile 64x64 score blocks are diagonal blocks of one banded 512x512
    score matrix S = Q K^T (band |i-j| <= 63).
  * With E = exp(S - rowmax), Cm[j,t] = m_t[j-t] (banded), the full
    tile-softmax + overlap-add + count-divide collapses to
        Z = E @ Cm;  W = bandmask/(counts * Z);  U = W @ Cm^T;
        out = (E * U) @ V
    i.e. three extra banded 512x512 matmuls instead of 449 gathered
    attentions.
  * bk drops exactly: it shifts each row's scores by a constant, which the
    rowmax-subtracted softmax cancels bit-for-bit.

Sharding: 8 cores = 2 batches x 4 row-chunks of 128 output rows. Each core
computes its 128 rows end-to-end from a 256-column band of the input (no
cross-core communication); host slices/pads inputs and concatenates outputs.

Precision plan: x and the three weight matrices ship as fp16 (halves the
DMA, which is the bottleneck at ~210 GB/s/core); all matmul products
accumulate in fp32 PSUM. The score/softmax chain (Q^T, K^T, S, E, Cm, W, U)
stays in float32r (~13-bit mantissa; fp16 E would underflow to subnormals
whenever a row's in-band max sits ~16 below its window max). The value path
(V, A, out-matmul) is fp16, where rounding only mixes linearly.

Per-core inputs are packed host-side into two blobs laid out exactly as
their SBUF destinations, DMA'd in priority-chained groups (2 parallel
queues per group) so compute starts after the first ~1 MB.
"""

import os
import sys

import numpy as np

for _p in ("/opt/trn_rl_repo",):
    if _p not in sys.path and os.path.isdir(_p):
        sys.path.insert(0, _p)

B, N, C, W = 2, 512, 512, 64
T = N - W + 1          # 449 tiles
RCH = 128              # output rows per core
NCORES = 8
BAND = 256             # per-core j/t band width (columns [r0-64, r0+192))
KC = C // 128          # 4 contraction chunks
JC = BAND // 128       # 2 band chunks

# blob16 layout (fp16 elements per partition)
OFF_XT = 0                       # [128, 4, 256]
OFF_WQT = OFF_XT + KC * BAND     # [128, 4, 512]
OFF_MISC = OFF_WQT + KC * C      # p0 rows: bq [512] | bv [512] | ones [128]
END16_G1 = OFF_MISC + 2 * C + 128
OFF_WKT = END16_G1               # [128, 4, 512]
END16_G2 = OFF_WKT + KC * C
OFF_WVT = END16_G2               # [128, 4, 512]
F16 = OFF_WVT + KC * C

# blob32 layout (fp32 elements per partition)
OFF_CM = 0                       # [128, 2, 256]
OFF_CMT = OFF_CM + JC * BAND
OFF_MW = OFF_CMT + JC * BAND     # [128, 2, 128]
OFF_ID = OFF_MW + JC * RCH       # [128, 128]
F32 = OFF_ID + 128

_CACHE = {}


def _slim_drain_and_barrier(self, tick_clock, wait_clock):
    """Cheaper TileContext exit. Every compute op in this kernel feeds the
    output DMA, so the final drain only needs to cover DMA-queue completion
    (not the full 27-proc global clock, whose multi-wait split costs an
    ~10us EVSEM butterfly). Engines are then synced with one sem-only
    barrier and the semaphores reset for NEFF re-executability."""
    from concourse.vector_clock import ScopedClock, VectorClock
    from concourse.tile_scheduler import dmasw_start_idx, N_PROCS

    g = tick_clock.global_clock
    dma_clock = VectorClock()
    for idx in range(dmasw_start_idx, N_PROCS):
        t = g.peek_next(idx) - 1
        if t > 0:
            dma_clock.require_at_least(idx, t)
    drain_inst = self.nc.sync.drain()
    wait_clock.add_sem_waits(drain_inst.ins, ScopedClock({None: dma_clock}))
    self.nc.all_engine_barrier(sem_only=True)
    popped = self.nc._tile_sem_poison_stack.pop()
    assert popped is self._sem_poison
    self.nc.clear_and_free_semaphores(list(self.sems.allocated().values()))


def _build_program():
    import concourse.bacc as bacc
    import concourse.mybir as mybir
    import concourse.tile as tile

    fp32 = mybir.dt.float32
    fp16 = mybir.dt.float16
    # Bass's preamble ends with a full all-engine barrier (drains + EVSEM,
    # ~3us); a sem-only barrier gives the same cross-engine ordering for the
    # preamble const memsets at a fraction of the cost.
    orig_aeb = bacc.Bacc.all_engine_barrier

    def _sem_only_aeb(self, *, sem_only=False):
        return orig_aeb(self, sem_only=True)

    bacc.Bacc.all_engine_barrier = _sem_only_aeb
    try:
        nc = bacc.Bacc("TRN2", target_bir_lowering=False, debug=False)
    finally:
        bacc.Bacc.all_engine_barrier = orig_aeb

    b16_d = nc.declare_dram_parameter("blob16", [128, F16], fp16, isOutput=False)
    b32_d = nc.declare_dram_parameter("blob32", [128, F32], fp32, isOutput=False)
    out_d = nc.declare_dram_parameter("out", [RCH, C], fp32, isOutput=True)

    orig_dab = tile.TileContext._drain_and_barrier
    tile.TileContext._drain_and_barrier = _slim_drain_and_barrier
    try:
        _emit_body(nc, tile, mybir, b16_d, b32_d, out_d)
    finally:
        tile.TileContext._drain_and_barrier = orig_dab

    nc.compile()
    return nc


def _emit_body(nc, tile, mybir, b16_d, b32_d, out_d):
    from concourse.tile_rust import add_dep_helper

    fp32 = mybir.dt.float32
    fp32r = mybir.dt.float32r
    fp16 = mybir.dt.float16

    with tile.TileContext(nc) as tc:
        with (
            tc.tile_pool(name="consts", bufs=1) as consts,
            tc.tile_pool(name="work", bufs=1) as work,
            tc.tile_pool(name="psum", bufs=1, space="PSUM") as psum,
        ):
            b16 = consts.tile([128, F16], fp16)
            b32 = consts.tile([128, F32], fp32r)
            b32_src = b32_d[:].bitcast(fp32r)
            # Priority-chained DMA groups; 2 parallel queues per group
            # (a single HWDGE queue tops out around ~200 GB/s).
            groups = [
                (b16, b16_d[:], 0, END16_G1),
                (b16, b16_d[:], END16_G1, END16_G2),
                (b32, b32_src, 0, F32),
                (b16, b16_d[:], END16_G2, F16),
            ]
            prev_group = []
            for dst, src, lo, hi in groups:
                npc = 4
                cuts = [lo + (hi - lo) * i // npc for i in range(npc + 1)]
                cur_group = []
                for a, b in zip(cuts, cuts[1:]):
                    if a == b:
                        continue
                    d = nc.sync.dma_start(out=dst[:, a:b], in_=src[:, a:b])
                    for p in prev_group:
                        add_dep_helper(d.ins, p.ins, True,
                                       "input DMA priority chain")
                    cur_group.append(d)
                prev_group = cur_group

            xt_sb = b16[:, OFF_XT:OFF_XT + KC * BAND].rearrange(
                "p (k j) -> p k j", k=KC)
            wqt_sb = b16[:, OFF_WQT:OFF_WQT + KC * C].rearrange(
                "p (k j) -> p k j", k=KC)
            wkt_sb = b16[:, OFF_WKT:OFF_WKT + KC * C].rearrange(
                "p (k j) -> p k j", k=KC)
            wvt_sb = b16[:, OFF_WVT:OFF_WVT + KC * C].rearrange(
                "p (k j) -> p k j", k=KC)
            bqr_sb = b16[0:1, OFF_MISC:OFF_MISC + C]
            bvr_sb = b16[0:1, OFF_MISC + C:OFF_MISC + 2 * C]
            ones1 = b16[0:1, OFF_MISC + 2 * C:OFF_MISC + 2 * C + 128]
            cm_sb = b32[:, OFF_CM:OFF_CM + JC * BAND].rearrange(
                "p (k t) -> p k t", k=JC)
            cmt_sb = b32[:, OFF_CMT:OFF_CMT + JC * BAND].rearrange(
                "p (k j) -> p k j", k=JC)
            mw_sb = b32[:, OFF_MW:OFF_MW + JC * RCH].rearrange(
                "p (k r) -> p k r", k=JC)
            ident = b32[:, OFF_ID:OFF_ID + 128]

            # ---- projections (fp16 inputs, fp32 PSUM accumulation) ----
            # Q rows [r 128, c 512] (+bq via rank-1 ones matmul), then
            # transpose to QT chunks [c 128, r 128] in fp32r
            ps_qrow = psum.tile([128, C], fp32, tag="ps_big", bufs=2)
            for k in range(KC):
                nc.tensor.matmul(
                    ps_qrow,
                    lhsT=xt_sb[:, k, 64:64 + RCH],
                    rhs=wqt_sb[:, k, :],
                    start=(k == 0),
                    stop=False,
                )
            nc.tensor.matmul(
                ps_qrow, lhsT=ones1, rhs=bqr_sb, start=False, stop=True,
            )
            q_sb = work.tile([128, C], fp32r)
            nc.vector.tensor_copy(out=q_sb, in_=ps_qrow)
            qt_sb = work.tile([128, KC, RCH], fp32r)
            for m in range(KC):
                ps_t = psum.tile([128, RCH], fp32r, tag="ps_t", bufs=2)
                nc.tensor.transpose(
                    ps_t, q_sb[:, m * 128:(m + 1) * 128], ident
                )
                nc.vector.tensor_copy(out=qt_sb[:, m, :], in_=ps_t)

            # KT[m][c_out 128, j 256]  (bk dropped: softmax-invariant)
            kt_sb = work.tile([128, KC, BAND], fp32r)
            for m in range(KC):
                ps_k = psum.tile([128, BAND], fp32, tag="ps_k", bufs=1)
                for k in range(KC):
                    nc.tensor.matmul(
                        ps_k,
                        lhsT=wkt_sb[:, k, m * 128:(m + 1) * 128],
                        rhs=xt_sb[:, k, :],
                        start=(k == 0),
                        stop=(k == KC - 1),
                    )
                nc.vector.tensor_copy(out=kt_sb[:, m, :], in_=ps_k)

            # V[jc][j 128, c 512] (+bv via rank-1 ones matmul), fp16
            v_sb = work.tile([128, JC, C], fp16)
            for jc in range(JC):
                ps_v = psum.tile([128, C], fp32, tag="ps_big", bufs=2)
                for k in range(KC):
                    nc.tensor.matmul(
                        ps_v,
                        lhsT=xt_sb[:, k, jc * 128:(jc + 1) * 128],
                        rhs=wvt_sb[:, k, :],
                        start=(k == 0),
                        stop=False,
                    )
                nc.tensor.matmul(
                    ps_v, lhsT=ones1, rhs=bvr_sb, start=False, stop=True,
                )
                nc.vector.tensor_copy(out=v_sb[:, jc, :], in_=ps_v)

            # ---- scores and softmax numerator (fp32r) ----
            ps_s = psum.tile([128, BAND], fp32, tag="ps_s", bufs=1)
            for k in range(KC):
                nc.tensor.matmul(
                    ps_s,
                    lhsT=qt_sb[:, k, :],
                    rhs=kt_sb[:, k, :],
                    start=(k == 0),
                    stop=(k == KC - 1),
                )
            negmax = work.tile([128, 1], fp32)
            nc.vector.reduce_max(
                negmax, ps_s, axis=mybir.AxisListType.X, negate=True
            )
            e_sb = work.tile([128, BAND], fp32r)
            nc.scalar.activation(
                out=e_sb, in_=ps_s,
                func=mybir.ActivationFunctionType.Exp,
                bias=negmax, scale=1.0,
            )

            # E^T chunks [j 128, r 128]
            et_sb = work.tile([128, JC, RCH], fp32r)
            for jc in range(JC):
                ps_t = psum.tile([128, RCH], fp32r, tag="ps_t", bufs=2)
                nc.tensor.transpose(
                    ps_t, e_sb[:, jc * 128:(jc + 1) * 128], ident
                )
                nc.vector.tensor_copy(out=et_sb[:, jc, :], in_=ps_t)

            # Z'[t 128, r 128] = sum_j Cm[j,t] E'[j,r];  W' = maskw / Z'
            w_sb = work.tile([128, JC, RCH], fp32r)
            for tch in range(JC):
                ps_z = psum.tile([128, RCH], fp32, tag="ps_zu", bufs=2)
                for jc in range(JC):
                    nc.tensor.matmul(
                        ps_z,
                        lhsT=cm_sb[:, jc, tch * 128:(tch + 1) * 128],
                        rhs=et_sb[:, jc, :],
                        start=(jc == 0),
                        stop=(jc == JC - 1),
                    )
                rz = work.tile([128, RCH], fp32, tag="rz", bufs=2)
                nc.vector.reciprocal(out=rz, in_=ps_z)
                nc.vector.tensor_mul(
                    w_sb[:, tch, :], rz, mw_sb[:, tch, :].bitcast(fp32)
                )

            # U'[j 128, r 128] = sum_t Cm^T[t,j] W'[t,r];  A' = E' * U'
            a_sb = work.tile([128, JC, RCH], fp16)
            for jc in range(JC):
                ps_u = psum.tile([128, RCH], fp32, tag="ps_zu", bufs=2)
                for tch in range(JC):
                    nc.tensor.matmul(
                        ps_u,
                        lhsT=cmt_sb[:, tch, jc * 128:(jc + 1) * 128],
                        rhs=w_sb[:, tch, :],
                        start=(tch == 0),
                        stop=(tch == JC - 1),
                    )
                nc.vector.tensor_mul(
                    a_sb[:, jc, :], ps_u, et_sb[:, jc, :].bitcast(fp32)
                )

            # out rows [r 128, c 512] = sum_j A'[j,r]^T V[j,c]  (fp16)
            ps_o = psum.tile([128, C], fp32, tag="ps_big", bufs=2)
            for jc in range(JC):
                nc.tensor.matmul(
                    ps_o,
                    lhsT=a_sb[:, jc, :],
                    rhs=v_sb[:, jc, :],
                    start=(jc == 0),
                    stop=(jc == JC - 1),
                )
            o_sb = work.tile([128, C], fp32)
            nc.vector.tensor_copy(out=o_sb, in_=ps_o)
            nc.sync.dma_start(out=out_d[:], in_=o_sb)


def _pack128(arr):
    """[n*128, f] row-chunked -> [128, n*f] (chunk-major along free axis)."""
    n = arr.shape[0] // 128
    return np.ascontiguousarray(
        arr.reshape(n, 128, -1).transpose(1, 0, 2).reshape(128, -1)
    )


def _host_prep(image_features, Wq, bq, Wk, bk, Wv, bv, sample_idx):
    """Build the 8 per-core input blobs (pure index/layout work)."""
    x = np.asarray(image_features, np.float32)
    sample_idx = np.asarray(sample_idx)

    # per-tile multiplicities -> banded count matrix Cm[j, t] = m_t[j - t]
    mod = (sample_idx % W).astype(np.int64)                  # [T, S]
    m = np.zeros((T, W), np.float32)
    np.add.at(m, (np.arange(T)[:, None], mod), 1.0)
    m += 1.0
    Cm = np.zeros((N, N), np.float32)
    rows = np.arange(T)
    for w in range(W):
        Cm[rows + w, rows] = m[:, w]

    pos = np.arange(N)
    counts = (np.minimum(pos, N - W) - np.maximum(pos - W + 1, 0) + 1)

    # padded versions for uniform band slicing
    XTp = np.zeros((B, C, N + 2 * 64), np.float16)
    for b in range(B):
        XTp[b, :, 64:64 + N] = x[b].T.astype(np.float16)
    Cmp = np.zeros((N + 2 * 64, N + 2 * 64), np.float32)
    Cmp[64:64 + N, 64:64 + N] = Cm

    wqt_p = _pack128(np.asarray(Wq, np.float32).T.astype(np.float16))
    wkt_p = _pack128(np.asarray(Wk, np.float32).T.astype(np.float16))
    wvt_p = _pack128(np.asarray(Wv, np.float32).T.astype(np.float16))

    in_maps = []
    for core in range(NCORES):
        b, rc = divmod(core, NCORES // B)
        r0 = rc * RCH
        xt = XTp[b, :, r0:r0 + BAND]
        cm = np.ascontiguousarray(Cmp[r0:r0 + BAND, r0:r0 + BAND])
        # all-zero columns (padded t) would give Z=0 -> 1/0*mask = NaN on
        # device; a diagonal 1 keeps Z finite there and is masked out of W
        zero_cols = ~cm.any(axis=0)
        cm[zero_cols, zero_cols] = 1.0
        tl = np.arange(BAND)
        rl = np.arange(RCH)
        tg = r0 - 64 + tl
        rg = r0 + rl
        d = rg[None, :] - tg[:, None]
        valid = (d >= 0) & (d <= W - 1) & (tg[:, None] >= 0) & (tg[:, None] <= T - 1)
        maskw = np.where(
            valid, 1.0 / counts[rg][None, :], 0.0
        ).astype(np.float32)

        b16 = np.zeros((128, F16), np.float16)
        b16[:, OFF_XT:OFF_XT + KC * BAND] = _pack128(xt)
        b16[:, OFF_WQT:OFF_WQT + KC * C] = wqt_p
        b16[:, OFF_WKT:OFF_WKT + KC * C] = wkt_p
        b16[:, OFF_WVT:OFF_WVT + KC * C] = wvt_p
        b16[0, OFF_MISC:OFF_MISC + C] = np.asarray(bq, np.float32)
        b16[0, OFF_MISC + C:OFF_MISC + 2 * C] = np.asarray(bv, np.float32)
        b16[0, OFF_MISC + 2 * C:OFF_MISC + 2 * C + 128] = 1.0

        b32 = np.zeros((128, F32), np.float32)
        b32[:, OFF_CM:OFF_CM + JC * BAND] = _pack128(cm)
        b32[:, OFF_CMT:OFF_CMT + JC * BAND] = _pack128(
            np.ascontiguousarray(cm.T)
        )
        b32[:, OFF_MW:OFF_MW + JC * RCH] = _pack128(maskw)
        b32[:, OFF_ID:OFF_ID + 128] = np.eye(128, dtype=np.float32)
        in_maps.append({"blob16": b16, "blob32": b32})
    return in_maps


def run_on_cores(in_maps, trace=False, trace_cores=None):
    from concourse.bass_utils import run_bass_kernel_spmd

    if "nc" not in _CACHE:
        _CACHE["nc"] = _build_program()
    nc = _CACHE["nc"]
    return run_bass_kernel_spmd(
        nc, in_maps, list(range(NCORES)), trace=trace,
        trace_cores=(trace_cores or [0]) if trace else None,
    )


def kernel(image_features, Wq, bq, Wk, bk, Wv, bv, sample_idx):
    in_maps = _host_prep(image_features, Wq, bq, Wk, bk, Wv, bv, sample_idx)
    res = run_on_cores(in_maps, trace=False)
    out = np.empty((B, N, C), np.float32)
    for core in range(NCORES):
        b, rc = divmod(core, NCORES // B)
        out[b, rc * RCH:(rc + 1) * RCH, :] = res.results[core]["out"]
    return out


# revision 29
# speedup vs baseline: 1.6978x; 1.0531x over previous
"""Trainium2 Bass kernel for ConsistentSelfAttentionTile.

Reference semantics: T=449 overlapping 64-token tiles; each tile attends to
352 KV tokens = 288 sampled (from a 9x replication of the tile) + the tile
itself; outputs overlap-add, then divide by overlap counts.

Algebraic collapse used here (verified to ~1e-6 rel vs the jax reference):
  * rep[:, idx, :] == tile[:, idx % 64, :], so the sampled KV tokens are tile
    rows with integer multiplicities m_t[w] = 1 + #{s : idx[t,s] % 64 == w}.
  * Per-tile Q/K/V are slices of the full-sequence projections, so all
    per-tile 64x64 score blocks are diagonal blocks of one banded 512x512
    score matrix S = Q K^T (band |i-j| <= 63).
  * With E = exp(S - rowmax), Cm[j,t] = m_t[j-t] (banded), the full
    tile-softmax + overlap-add + count-divide collapses to
        Z = E @ Cm;  W = bandmask/(counts * Z);  U = W @ Cm^T;
        out = (E * U) @ V
    i.e. three extra banded 512x512 matmuls instead of 449 gathered
    attentions.
  * bk drops exactly: it shifts each row's scores by a constant, which the
    rowmax-subtracted softmax cancels bit-for-bit.

Sharding: 8 cores = 2 batches x 4 row-chunks of 128 output rows. Each core
computes its 128 rows end-to-end from a 256-column band of the input (no
cross-core communication); host slices/pads inputs and concatenates outputs.

Precision plan: x and the three weight matrices ship as fp16 (halves the
DMA, which is the bottleneck at ~210 GB/s/core); all matmul products
accumulate in fp32 PSUM. The score/softmax chain (Q^T, K^T, S, E, Cm, W, U)
stays in float32r (~13-bit mantissa; fp16 E would underflow to subnormals
whenever a row's in-band max sits ~16 below its window max). The value path
(V, A, out-matmul) is fp16, where rounding only mixes linearly.

Per-core inputs are packed host-side into two blobs laid out exactly as
their SBUF destinations, DMA'd in priority-chained groups (2 parallel
queues per group) so compute starts after the first ~1 MB.
"""

import os
import sys

import numpy as np

try:
    import ml_dtypes
except ImportError:
    ml_dtypes = None

for _p in ("/opt/trn_rl_repo",):
    if _p not in sys.path and os.path.isdir(_p):
        sys.path.insert(0, _p)

B, N, C, W = 2, 512, 512, 64
T = N - W + 1          # 449 tiles
RCH = 128              # output rows per core
NCORES = 8
BAND = 256             # per-core j/t band width (columns [r0-64, r0+192))
KC = C // 128          # 4 contraction chunks
JC = BAND // 128       # 2 band chunks

# blob16 layout (2-byte elements per partition; fp16 except the bf16 Cm
# segments, which are bitcast views)
OFF_XT = 0                       # [128, 4, 256] fp16
OFF_WQT = OFF_XT + KC * BAND     # [128, 4, 512] fp16
OFF_MISC = OFF_WQT + KC * C      # p0 rows: bq [512] | bv [512] | ones [128]
END16_G1 = OFF_MISC + 2 * C + 128
OFF_WKT = END16_G1               # [128, 4, 512] fp16
END16_G2 = OFF_WKT + KC * C
OFF_CM = END16_G2                # [128, 2, 256] bf16 (count ints: exact)
OFF_CMT = OFF_CM + JC * BAND     # [128, 2, 256] bf16
OFF_ID16 = OFF_CMT + JC * BAND   # [128, 128] bf16 identity
END16_G3 = OFF_ID16 + 128
OFF_WVT = END16_G3               # [128, 4, 512] fp16
F16 = OFF_WVT + KC * C

# blob32 layout (fp32 elements per partition; DMA'd with DMA group 1)
OFF_MW = 0                       # [128, 2, 128]
OFF_ID = OFF_MW + JC * RCH       # [128, 128] identity (fp32r via bitcast)
F32 = OFF_ID + 128

_CACHE = {}


def _slim_drain_and_barrier(self, tick_clock, wait_clock):
    """Cheaper TileContext exit. Every compute op in this kernel feeds the
    output DMA, so the final drain only needs to cover DMA-queue completion
    (not the full 27-proc global clock, whose multi-wait split costs an
    ~10us EVSEM butterfly). Engines are then synced with one sem-only
    barrier and the semaphores reset for NEFF re-executability."""
    from concourse.vector_clock import ScopedClock, VectorClock
    from concourse.tile_scheduler import dmasw_start_idx, N_PROCS

    g = tick_clock.global_clock
    dma_clock = VectorClock()
    for idx in range(dmasw_start_idx, N_PROCS):
        t = g.peek_next(idx) - 1
        if t > 0:
            dma_clock.require_at_least(idx, t)
    drain_inst = self.nc.sync.drain()
    wait_clock.add_sem_waits(drain_inst.ins, ScopedClock({None: dma_clock}))
    self.nc.all_engine_barrier(sem_only=True)
    popped = self.nc._tile_sem_poison_stack.pop()
    assert popped is self._sem_poison
    self.nc.clear_and_free_semaphores(list(self.sems.allocated().values()))


def _build_program():
    import concourse.bacc as bacc
    import concourse.mybir as mybir
    import concourse.tile as tile

    fp32 = mybir.dt.float32
    fp16 = mybir.dt.float16
    # Bass's preamble ends with a full all-engine barrier (drains + EVSEM,
    # ~3-5us with the PE's first-IRAM-block stall). Our kernel never reads
    # the preamble's const APs and all real cross-engine deps are Tile
    # semaphores, so skip it: engines start independently and the input DMA
    # issues ~5us earlier.
    orig_aeb = bacc.Bacc.all_engine_barrier

    def _noop_aeb(self, *, sem_only=False):
        return None

    bacc.Bacc.all_engine_barrier = _noop_aeb
    try:
        nc = bacc.Bacc("TRN2", target_bir_lowering=False, debug=False)
    finally:
        bacc.Bacc.all_engine_barrier = orig_aeb

    b16_d = nc.declare_dram_parameter("blob16", [128, F16], fp16, isOutput=False)
    b32_d = nc.declare_dram_parameter("blob32", [128, F32], fp32, isOutput=False)
    out_d = nc.declare_dram_parameter("out", [RCH, C], fp32, isOutput=True)

    orig_dab = tile.TileContext._drain_and_barrier
    tile.TileContext._drain_and_barrier = _slim_drain_and_barrier
    try:
        _emit_body(nc, tile, mybir, b16_d, b32_d, out_d)
    finally:
        tile.TileContext._drain_and_barrier = orig_dab

    nc.compile()
    return nc


def _emit_body(nc, tile, mybir, b16_d, b32_d, out_d):
    from concourse.tile_rust import add_dep_helper

    fp32 = mybir.dt.float32
    fp32r = mybir.dt.float32r
    fp16 = mybir.dt.float16

    with tile.TileContext(nc) as tc:
        with (
            tc.tile_pool(name="consts", bufs=1) as consts,
            tc.tile_pool(name="work", bufs=1) as work,
            tc.tile_pool(name="psum", bufs=1, space="PSUM") as psum,
        ):
            b16 = consts.tile([128, F16], fp16)
            b32 = consts.tile([128, F32], fp32r)
            # Priority-chained DMA groups, 3 parallel queues per group (a
            # single HWDGE queue tops out ~200 GB/s), issued alternately
            # from the two HWDGE-capable engines (sync, scalar) since each
            # PSEUDO_DMA issue costs ~0.6us of engine time. Chaining is one
            # dep per piece (index-matched) to bound the evsem-split cost.
            groups = [
                [(b16, b16_d[:], 0, END16_G1, 3),
                 (b32, b32_d[:].bitcast(fp32r), 0, F32, 1)],
                [(b16, b16_d[:], END16_G1, END16_G2, 3)],
                [(b16, b16_d[:], END16_G2, END16_G3, 2)],
                [(b16, b16_d[:], END16_G3, F16, 3)],
            ]
            issuers = [nc.sync, nc.scalar]
            prev_group = []
            n_issued = 0
            for group in groups:
                cur_group = []
                for dst, src, lo, hi, npc in group:
                    cuts = [lo + (hi - lo) * i // npc
                            for i in range(npc + 1)]
                    for a, b in zip(cuts, cuts[1:]):
                        if a == b:
                            continue
                        eng = issuers[n_issued % len(issuers)]
                        n_issued += 1
                        d = eng.dma_start(out=dst[:, a:b], in_=src[:, a:b])
                        if prev_group:
                            p = prev_group[len(cur_group) % len(prev_group)]
                            add_dep_helper(d.ins, p.ins, True,
                                           "input DMA priority chain")
                        cur_group.append(d)
                prev_group = cur_group

            xt_sb = b16[:, OFF_XT:OFF_XT + KC * BAND].rearrange(
                "p (k j) -> p k j", k=KC)
            wqt_sb = b16[:, OFF_WQT:OFF_WQT + KC * C].rearrange(
                "p (k j) -> p k j", k=KC)
            wkt_sb = b16[:, OFF_WKT:OFF_WKT + KC * C].rearrange(
                "p (k j) -> p k j", k=KC)
            wvt_sb = b16[:, OFF_WVT:OFF_WVT + KC * C].rearrange(
                "p (k j) -> p k j", k=KC)
            bqr_sb = b16[0:1, OFF_MISC:OFF_MISC + C]
            bvr_sb = b16[0:1, OFF_MISC + C:OFF_MISC + 2 * C]
            ones1 = b16[0:1, OFF_MISC + 2 * C:OFF_MISC + 2 * C + 128]
            bf16 = mybir.dt.bfloat16
            cm_sb = b16[:, OFF_CM:OFF_CM + JC * BAND].bitcast(bf16).rearrange(
                "p (k t) -> p k t", k=JC)
            cmt_sb = b16[:, OFF_CMT:OFF_CMT + JC * BAND].bitcast(
                bf16).rearrange("p (k j) -> p k j", k=JC)
            mw_sb = b32[:, OFF_MW:OFF_MW + JC * RCH].bitcast(
                fp32).rearrange("p (k r) -> p k r", k=JC)
            ident = b32[:, OFF_ID:OFF_ID + 128]
            ident16 = b16[:, OFF_ID16:OFF_ID16 + 128].bitcast(bf16)

            # ---- projections (fp16 inputs, fp32 PSUM accumulation) ----
            # Q rows [r 128, c 512] (+bq via rank-1 ones matmul), then
            # transpose to QT chunks [c 128, r 128] in fp32r
            ps_qrow = psum.tile([128, C], fp32, tag="ps_big", bufs=2)
            for k in range(KC):
                nc.tensor.matmul(
                    ps_qrow,
                    lhsT=xt_sb[:, k, 64:64 + RCH],
                    rhs=wqt_sb[:, k, :],
                    start=(k == 0),
                    stop=False,
                )
            nc.tensor.matmul(
                ps_qrow, lhsT=ones1, rhs=bqr_sb, start=False, stop=True,
            )
            q_sb = work.tile([128, C], fp32r)
            nc.vector.tensor_copy(out=q_sb, in_=ps_qrow)
            qt_sb = work.tile([128, KC, RCH], fp32r)
            for m in range(KC):
                ps_t = psum.tile([128, RCH], fp32r, tag="ps_t", bufs=2)
                nc.tensor.transpose(
                    ps_t, q_sb[:, m * 128:(m + 1) * 128], ident
                )
                nc.vector.tensor_copy(out=qt_sb[:, m, :], in_=ps_t)

            # KT[m][c_out 128, j 256]  (bk dropped: softmax-invariant)
            kt_sb = work.tile([128, KC, BAND], fp32r)
            for m in range(KC):
                ps_k = psum.tile([128, BAND], fp32, tag="ps_k", bufs=1)
                for k in range(KC):
                    nc.tensor.matmul(
                        ps_k,
                        lhsT=wkt_sb[:, k, m * 128:(m + 1) * 128],
                        rhs=xt_sb[:, k, :],
                        start=(k == 0),
                        stop=(k == KC - 1),
                    )
                nc.vector.tensor_copy(out=kt_sb[:, m, :], in_=ps_k)

            # V[jc][j 128, c 512] (+bv via rank-1 ones matmul), fp16
            v_sb = work.tile([128, JC, C], fp16)
            for jc in range(JC):
                ps_v = psum.tile([128, C], fp32, tag="ps_big", bufs=2)
                for k in range(KC):
                    nc.tensor.matmul(
                        ps_v,
                        lhsT=xt_sb[:, k, jc * 128:(jc + 1) * 128],
                        rhs=wvt_sb[:, k, :],
                        start=(k == 0),
                        stop=False,
                    )
                nc.tensor.matmul(
                    ps_v, lhsT=ones1, rhs=bvr_sb, start=False, stop=True,
                )
                nc.vector.tensor_copy(out=v_sb[:, jc, :], in_=ps_v)

            # ---- scores and softmax numerator (fp32r) ----
            ps_s = psum.tile([128, BAND], fp32, tag="ps_s", bufs=1)
            for k in range(KC):
                nc.tensor.matmul(
                    ps_s,
                    lhsT=qt_sb[:, k, :],
                    rhs=kt_sb[:, k, :],
                    start=(k == 0),
                    stop=(k == KC - 1),
                )
            negmax = work.tile([128, 1], fp32)
            nc.vector.reduce_max(
                negmax, ps_s, axis=mybir.AxisListType.X, negate=True
            )
            e_sb = work.tile([128, BAND], bf16)
            nc.scalar.activation(
                out=e_sb, in_=ps_s,
                func=mybir.ActivationFunctionType.Exp,
                bias=negmax, scale=1.0,
            )

            # E^T chunks [j 128, r 128]
            et_sb = work.tile([128, JC, RCH], bf16)
            for jc in range(JC):
                ps_e = psum.tile([128, RCH], bf16, tag="ps_t", bufs=2)
                nc.tensor.transpose(
                    ps_e, e_sb[:, jc * 128:(jc + 1) * 128], ident16
                )
                nc.vector.tensor_copy(out=et_sb[:, jc, :], in_=ps_e)

            # Z'[t 128, r 128] = sum_j Cm[j,t] E'[j,r];  W' = maskw / Z'
            w_sb = work.tile([128, JC, RCH], bf16)
            for tch in range(JC):
                ps_z = psum.tile([128, RCH], fp32, tag="ps_zu", bufs=2)
                for jc in range(JC):
                    nc.tensor.matmul(
                        ps_z,
                        lhsT=cm_sb[:, jc, tch * 128:(tch + 1) * 128],
                        rhs=et_sb[:, jc, :],
                        start=(jc == 0),
                        stop=(jc == JC - 1),
                    )
                rz = work.tile([128, RCH], fp32, tag="rz", bufs=2)
                nc.vector.reciprocal(out=rz, in_=ps_z)
                nc.vector.tensor_mul(
                    w_sb[:, tch, :], rz, mw_sb[:, tch, :]
                )

            # U'[j 128, r 128] = sum_t Cm^T[t,j] W'[t,r];  A' = E' * U'
            a_sb = work.tile([128, JC, RCH], fp16)
            for jc in range(JC):
                ps_u = psum.tile([128, RCH], fp32, tag="ps_zu", bufs=2)
                for tch in range(JC):
                    nc.tensor.matmul(
                        ps_u,
                        lhsT=cmt_sb[:, tch, jc * 128:(jc + 1) * 128],
                        rhs=w_sb[:, tch, :],
                        start=(tch == 0),
                        stop=(tch == JC - 1),
                    )
                nc.vector.tensor_mul(
                    a_sb[:, jc, :], ps_u, et_sb[:, jc, :]
                )

            # out rows [r 128, c 512] = sum_j A'[j,r]^T V[j,c]  (fp16)
            ps_o = psum.tile([128, C], fp32, tag="ps_big", bufs=2)
            for jc in range(JC):
                nc.tensor.matmul(
                    ps_o,
                    lhsT=a_sb[:, jc, :],
                    rhs=v_sb[:, jc, :],
                    start=(jc == 0),
                    stop=(jc == JC - 1),
                )
            o_sb = work.tile([128, C], fp32)
            nc.vector.tensor_copy(out=o_sb, in_=ps_o)
            nc.sync.dma_start(out=out_d[:], in_=o_sb)


def _pack128(arr):
    """[n*128, f] row-chunked -> [128, n*f] (chunk-major along free axis)."""
    n = arr.shape[0] // 128
    return np.ascontiguousarray(
        arr.reshape(n, 128, -1).transpose(1, 0, 2).reshape(128, -1)
    )


def _host_prep(image_features, Wq, bq, Wk, bk, Wv, bv, sample_idx):
    """Build the 8 per-core input blobs (pure index/layout work)."""
    x = np.asarray(image_features, np.float32)
    sample_idx = np.asarray(sample_idx)

    # per-tile multiplicities -> banded count matrix Cm[j, t] = m_t[j - t]
    mod = (sample_idx % W).astype(np.int64)                  # [T, S]
    m = np.zeros((T, W), np.float32)
    np.add.at(m, (np.arange(T)[:, None], mod), 1.0)
    m += 1.0
    Cm = np.zeros((N, N), np.float32)
    rows = np.arange(T)
    for w in range(W):
        Cm[rows + w, rows] = m[:, w]

    pos = np.arange(N)
    counts = (np.minimum(pos, N - W) - np.maximum(pos - W + 1, 0) + 1)

    # padded versions for uniform band slicing
    XTp = np.zeros((B, C, N + 2 * 64), np.float16)
    for b in range(B):
        XTp[b, :, 64:64 + N] = x[b].T.astype(np.float16)
    Cmp = np.zeros((N + 2 * 64, N + 2 * 64), np.float32)
    Cmp[64:64 + N, 64:64 + N] = Cm

    wqt_p = _pack128(np.asarray(Wq, np.float32).T.astype(np.float16))
    wkt_p = _pack128(np.asarray(Wk, np.float32).T.astype(np.float16))
    wvt_p = _pack128(np.asarray(Wv, np.float32).T.astype(np.float16))

    in_maps = []
    for core in range(NCORES):
        b, rc = divmod(core, NCORES // B)
        r0 = rc * RCH
        xt = XTp[b, :, r0:r0 + BAND]
        cm = np.ascontiguousarray(Cmp[r0:r0 + BAND, r0:r0 + BAND])
        # all-zero columns (padded t) would give Z=0 -> 1/0*mask = NaN on
        # device; a diagonal 1 keeps Z finite there and is masked out of W
        zero_cols = ~cm.any(axis=0)
        cm[zero_cols, zero_cols] = 1.0
        tl = np.arange(BAND)
        rl = np.arange(RCH)
        tg = r0 - 64 + tl
        rg = r0 + rl
        d = rg[None, :] - tg[:, None]
        valid = (d >= 0) & (d <= W - 1) & (tg[:, None] >= 0) & (tg[:, None] <= T - 1)
        maskw = np.where(
            valid, 1.0 / counts[rg][None, :], 0.0
        ).astype(np.float32)

        b16 = np.zeros((128, F16), np.float16)
        b16[:, OFF_XT:OFF_XT + KC * BAND] = _pack128(xt)
        b16[:, OFF_WQT:OFF_WQT + KC * C] = wqt_p
        b16[:, OFF_WKT:OFF_WKT + KC * C] = wkt_p
        b16[:, OFF_WVT:OFF_WVT + KC * C] = wvt_p
        b16[0, OFF_MISC:OFF_MISC + C] = np.asarray(bq, np.float32)
        b16[0, OFF_MISC + C:OFF_MISC + 2 * C] = np.asarray(bv, np.float32)
        b16[0, OFF_MISC + 2 * C:OFF_MISC + 2 * C + 128] = 1.0
        # Cm segments carry bf16 bits (count ints are exact in bf16);
        # written through a uint16 view of the fp16 buffer
        b16v = b16.view(np.uint16)
        b16v[:, OFF_CM:OFF_CM + JC * BAND] = _pack128(
            cm.astype(ml_dtypes.bfloat16)).view(np.uint16)
        b16v[:, OFF_CMT:OFF_CMT + JC * BAND] = _pack128(
            np.ascontiguousarray(cm.T).astype(ml_dtypes.bfloat16)
        ).view(np.uint16)

        b16v[:, OFF_ID16:OFF_ID16 + 128] = np.eye(
            128, dtype=ml_dtypes.bfloat16).view(np.uint16)

        b32 = np.zeros((128, F32), np.float32)
        b32[:, OFF_MW:OFF_MW + JC * RCH] = _pack128(maskw)
        b32[:, OFF_ID:OFF_ID + 128] = np.eye(128, dtype=np.float32)
        in_maps.append({"blob16": b16, "blob32": b32})
    return in_maps


def run_on_cores(in_maps, trace=False, trace_cores=None):
    from concourse.bass_utils import run_bass_kernel_spmd

    if "nc" not in _CACHE:
        _CACHE["nc"] = _build_program()
    nc = _CACHE["nc"]
    return run_bass_kernel_spmd(
        nc, in_maps, list(range(NCORES)), trace=trace,
        trace_cores=(trace_cores or [0]) if trace else None,
    )


def kernel(image_features, Wq, bq, Wk, bk, Wv, bv, sample_idx):
    in_maps = _host_prep(image_features, Wq, bq, Wk, bk, Wv, bv, sample_idx)
    res = run_on_cores(in_maps, trace=False)
    out = np.empty((B, N, C), np.float32)
    for core in range(NCORES):
        b, rc = divmod(core, NCORES // B)
        out[b, rc * RCH:(rc + 1) * RCH, :] = res.results[core]["out"]
    return out
